# revision 1
# baseline (speedup 1.0000x reference)
"""Trainium2 Bass kernel for nn_GNN_node_30279519437414 (GNN message passing).

Self-contained: takes FULL inputs, shards across 8 NeuronCores internally,
returns the FULL output.

Strategy (per the sharding hint):
  - Nodes are sharded contiguously across 8 cores (25000 inst + 6250 net each,
    re-ordered into a shard-major "table" node order).
  - Edges are partitioned by destination core; each core owns the aggregation
    for its 31250 nodes.
  - Per layer, a full copy of h (feature-major) is AllGathered so every core
    can compute the full "message table"  x' = dis * relu(h @ W + b)  locally
    (the deg^-1/2 source factor is folded into the table, the destination
    factor is applied per-node after aggregation - both factor out exactly).
  - Message passing is then pure DMA: dma_gather rows of x' by source id,
    dma_scatter_add them into agg by destination id.  Scatter calls are
    split into "rounds" with unique destinations per call (the HW CCE add
    loses updates on duplicate indices within one call) and serialized by
    Tile's WAW tracking on the agg tensor.
"""

import sys

sys.path.insert(0, "/opt/trn_rl_repo")

import numpy as np

NC = 8
D = 64
L = 3
EPS = 1e-5
CALL_MAX = 1024

_CACHE = {}


# ---------------------------------------------------------------------------
# host-side preprocessing
# ---------------------------------------------------------------------------

def _sizes(inputs):
    n_inst = inputs["x"].shape[0]
    n_net = inputs["x_net"].shape[0]
    assert n_inst % NC == 0 and n_net % NC == 0
    si, sn = n_inst // NC, n_net // NC
    shard = si + sn
    shard_p = ((shard + 127) // 128) * 128
    return n_inst, n_net, si, sn, shard, shard_p


def _ref_to_table(ids, n_inst, si, sn, shard_p):
    """Map reference node ids -> shard-major table row ids."""
    ids = np.asarray(ids, dtype=np.int64)
    is_net = ids >= n_inst
    inst_core = ids // max(si, 1)
    inst_loc = ids - inst_core * si
    r = ids - n_inst
    net_core = r // max(sn, 1)
    net_loc = r - net_core * sn
    out = np.where(
        is_net,
        net_core * shard_p + si + net_loc,
        inst_core * shard_p + inst_loc,
    )
    return out


def _edge_plan(edge_index, n_inst, n_net, si, sn, shard, shard_p, reg_tiles):
    """Build per-core gather/scatter index arrays + a common call table.

    The destination space is split into NREG regions of reg_tiles node-tiles
    each; every region gets its own agg tensor so scatter-add WAW chains are
    independent.  Within one call, destinations are unique (scatter "rounds")
    because the HW CCE add loses updates on duplicate indices in one call.

    Returns per direction: (calls, gidx[NC], sidx[NC], tot).
    calls: list of (offset, size, chunk, region), size % 128 == 0, common to
    all cores.  gidx: chunk-local source rows.  sidx: region-local destination
    rows (pads point at dump rows >= reg_tiles*128).
    """
    row = np.asarray(edge_index[0], dtype=np.int64)
    col = np.asarray(edge_index[1], dtype=np.int64)
    tab_row = _ref_to_table(row, n_inst, si, sn, shard_p)
    tab_col = _ref_to_table(col, n_inst, si, sn, shard_p)
    reg_rows = reg_tiles * 128
    nreg = (shard_p + reg_rows - 1) // reg_rows

    plans = []
    for (s_tab, t_tab) in ((tab_row, tab_col), (tab_col, tab_row)):
        core = t_tab // shard_p
        dst = t_tab % shard_p
        reg = dst // reg_rows
        dloc = dst - reg * reg_rows
        chunk = s_tab // shard_p
        src = s_tab % shard_p

        # sort by (core, chunk, region, dst); round = occurrence idx per dst
        o1 = np.lexsort((dloc, reg, chunk, core))
        c_s, ch_s, rg_s, d_s, s_s = (core[o1], chunk[o1], reg[o1], dloc[o1],
                                     src[o1])
        grp = (((c_s * NC + ch_s) * nreg) + rg_s) * reg_rows + d_s
        new_grp = np.empty(len(grp), dtype=bool)
        new_grp[0] = True
        np.not_equal(grp[1:], grp[:-1], out=new_grp[1:])
        gstart = np.flatnonzero(new_grp)
        gcnt = np.diff(np.r_[gstart, len(grp)])
        rnd = np.arange(len(grp)) - np.repeat(gstart, gcnt)

        rmax = int(rnd.max()) + 1
        # group id per edge: (chunk, region, round); count per (core, gid)
        gid = (ch_s * nreg + rg_s) * rmax + rnd
        ngid = NC * nreg * rmax
        cnt = np.zeros((NC, ngid), dtype=np.int64)
        np.add.at(cnt, (c_s, gid), 1)
        size_g = cnt.max(axis=0)                     # max over cores
        pad_g = ((size_g + 127) // 128) * 128

        # call table (common to all cores)
        calls = []
        offsets_g = np.zeros(ngid, dtype=np.int64)
        off = 0
        for s in range(NC):
            for r_ in range(nreg):
                for rr in range(rmax):
                    g = (s * nreg + r_) * rmax + rr
                    p = int(pad_g[g])
                    if p == 0:
                        continue
                    offsets_g[g] = off
                    k = 0
                    while k < p:
                        sz = min(CALL_MAX, p - k)
                        calls.append((off + k, sz, s, r_))
                        k += sz
                    off += p
        tot = off

        gidx = np.zeros((NC, tot), dtype=np.int16)
        sidx = np.empty((NC, tot), dtype=np.int16)
        pad_pattern = (reg_rows + (np.arange(tot) % 128)).astype(np.int16)
        sidx[:] = pad_pattern[None, :]

        # position of each edge inside its (core, gid) group
        grp2 = c_s * ngid + gid
        o3 = np.argsort(grp2, kind="stable")
        grp2_s = grp2[o3]
        new2 = np.empty(len(grp2_s), dtype=bool)
        new2[0] = True
        np.not_equal(grp2_s[1:], grp2_s[:-1], out=new2[1:])
        g2start = np.flatnonzero(new2)
        g2cnt = np.diff(np.r_[g2start, len(grp2_s)])
        pos_in_grp = np.arange(len(grp2_s)) - np.repeat(g2start, g2cnt)
        pos = np.empty(len(grp2_s), dtype=np.int64)
        pos[o3] = pos_in_grp
        dest = offsets_g[gid] + pos
        gidx[c_s, dest] = s_s.astype(np.int16)
        sidx[c_s, dest] = d_s.astype(np.int16)

        plans.append((calls, gidx, sidx, tot))
    return plans


def _wrap_idx_dram(arr):
    """[tot] int16 -> [128, tot//16] (16-partition wrap replicated x8)."""
    w = arr.reshape(-1, 16).T.copy()  # [16, tot/16]
    return np.ascontiguousarray(np.tile(w, (8, 1)))


def _prep(inputs):
    n_inst, n_net, si, sn, shard, shard_p = _sizes(inputs)
    N = n_inst + n_net
    ntab = shard_p * NC
    nt = shard_p // 128          # node tiles per shard
    gt = nt * NC                 # global node tiles

    f = lambda k: np.asarray(inputs[k], dtype=np.float32)
    edge_index = inputs["edge_index"]
    row = np.asarray(edge_index[0], dtype=np.int64)
    col = np.asarray(edge_index[1], dtype=np.int64)

    deg_f = (np.bincount(row, minlength=N) + 1).astype(np.float32)
    deg_r = (np.bincount(col, minlength=N) + 1).astype(np.float32)
    dis_f = deg_f ** -0.5
    dis_r = deg_r ** -0.5
    inv_f = (1.0 / deg_f).astype(np.float32)
    inv_r = (1.0 / deg_r).astype(np.float32)

    # reference-order -> table-order per-node arrays, padded with 1.0
    perm = np.empty(ntab, dtype=np.int64)  # table row -> ref id (pad -> 0)
    valid = np.zeros(ntab, dtype=bool)
    for c in range(NC):
        base = c * shard_p
        perm[base:base + si] = np.arange(c * si, (c + 1) * si)
        perm[base + si:base + si + sn] = n_inst + np.arange(c * sn, (c + 1) * sn)
        perm[base + si + sn:base + shard_p] = 0
        valid[base:base + si + sn] = True

    def tabize(a):
        t = a[perm].astype(np.float32)
        t[~valid] = 1.0
        return np.ascontiguousarray(t.reshape(gt, 128).T)  # [128, gt]

    disf_t = tabize(dis_f)
    disr_t = tabize(dis_r)
    invf_t = tabize(inv_f)
    invr_t = tabize(inv_r)

    reg_tiles = (nt + 3) // 4
    plans = _edge_plan(edge_index, n_inst, n_net, si, sn, shard, shard_p,
                       reg_tiles)

    # weights
    enc1_Wb = np.vstack([f("enc1_W"), f("enc1_b")[None, :]])      # [17, 128]
    net1_Wb = np.vstack([f("net1_W"), f("net1_b")[None, :]])      # [9, 64]
    enc2_W, enc2_b = f("enc2_W"), f("enc2_b")
    net2_W, net2_b = f("net2_W"), f("net2_b")
    conv_W, conv_b, conv_root = f("conv_W"), f("conv_b"), f("conv_root")
    re_W, re_b, re_root = f("re_W"), f("re_b"), f("re_root")
    ln_g, ln_b = f("ln_g"), f("ln_b")

    wcat = np.zeros((L, 65, 128), np.float32)
    wcat_root = np.zeros((L, 65, 128), np.float32)
    for l in range(L):
        wcat[l, :64, :64] = conv_W[l]
        wcat[l, :64, 64:] = re_W[l]
        wcat[l, 64, :64] = conv_b[l]
        wcat[l, 64, 64:] = re_b[l]
        wcat_root[l] = wcat[l]
        wcat_root[l, 64, :64] += conv_root[l]
        wcat_root[l, 64, 64:] += re_root[l]

    flags = {
        "enc2_bias": not np.allclose(enc2_b, 0.0),
        "net2_bias": not np.allclose(net2_b, 0.0),
        "ln_g": [not np.allclose(ln_g[l], 1.0) for l in range(L)],
        "ln_b": [not np.allclose(ln_b[l], 0.0) for l in range(L)],
    }

    # per-core inputs
    x = f("x")
    x_net = f("x_net")
    ones = np.ones
    per_core = []
    for c in range(NC):
        xT = np.vstack([x[c * si:(c + 1) * si].T,
                        ones((1, si), np.float32)])              # [17, si]
        xnT = np.vstack([x_net[c * sn:(c + 1) * sn].T,
                         ones((1, sn), np.float32)])             # [9, sn]
        d = {
            "xT": np.ascontiguousarray(xT),
            "xnT": np.ascontiguousarray(xnT),
            "disf_own": np.ascontiguousarray(disf_t[:, c * nt:(c + 1) * nt]),
            "disr_own": np.ascontiguousarray(disr_t[:, c * nt:(c + 1) * nt]),
            "invf_own": np.ascontiguousarray(invf_t[:, c * nt:(c + 1) * nt]),
            "invr_own": np.ascontiguousarray(invr_t[:, c * nt:(c + 1) * nt]),
            "gidx_f": _wrap_idx_dram(plans[0][1][c]),
            "sidx_f": _wrap_idx_dram(plans[0][2][c]),
            "gidx_r": _wrap_idx_dram(plans[1][1][c]),
            "sidx_r": _wrap_idx_dram(plans[1][2][c]),
            # shared tensors (replicated):
            "enc1_Wb": enc1_Wb, "enc2_W": np.ascontiguousarray(enc2_W),
            "enc2_b": enc2_b.reshape(64, 1),
            "net1_Wb": net1_Wb, "net2_W": np.ascontiguousarray(net2_W),
            "net2_b": net2_b.reshape(64, 1),
            "wcat": wcat, "wcat_root": wcat_root,
            "disf_all": disf_t, "disr_all": disr_t,
            "ln_g": np.ascontiguousarray(np.broadcast_to(ln_g[:, None, :], (L, 128, 64))),
            "ln_b": np.ascontiguousarray(np.broadcast_to(ln_b[:, None, :], (L, 128, 64))),
        }
        per_core.append(d)

    meta = {
        "n_inst": n_inst, "n_net": n_net, "si": si, "sn": sn,
        "shard": shard, "shard_p": shard_p, "nt": nt, "gt": gt,
        "calls_f": plans[0][0], "tot_f": plans[0][3],
        "calls_r": plans[1][0], "tot_r": plans[1][3],
        "reg_tiles": reg_tiles, "flags": flags,
    }
    return meta, per_core


# ---------------------------------------------------------------------------
# device program
# ---------------------------------------------------------------------------

def _patch_lane_assignment():
    """Make Tile's DMASW lane choice queue-aware (queue q owns lanes 2q/2q+1)
    so SWDGE-queue round-robin doesn't trip the lane<->queue lock."""
    import concourse.tile_sem_assignment as tsa
    import concourse.mybir as mybir
    import concourse.bass_isa as bass_isa
    if getattr(tsa.TileClockTick, "_q_aware", False):
        return
    orig = tsa.TileClockTick._assign_tick

    def _assign_tick(self, inst):
        if (isinstance(inst, tsa.DMAInst)
                and not isinstance(inst, bass_isa.UserSyncedRemoteDMADescs)
                and inst.engine == mybir.EngineType.Pool
                and self.swdge_sem_count == tsa.NUM_SWDGE_GLOBAL_SEMS):
            qn = getattr(inst, "queue_num", 0) or 0
            if not hasattr(self, "_q_rr"):
                self._q_rr = {}
            r = self._q_rr.get(qn, 0)
            self._q_rr[qn] = r ^ 1
            self.next_sw_dma_idx = (qn * 2 + r) % self.swdge_sem_count
        return orig(self, inst)

    tsa.TileClockTick._assign_tick = _assign_tick
    tsa.TileClockTick._q_aware = True


def _build(meta):
    import concourse.bass as bass
    import concourse.bacc as bacc
    import concourse.mybir as mybir
    from concourse import tile

    _patch_lane_assignment()

    dt = mybir.dt
    AF = mybir.ActivationFunctionType
    OP = mybir.AluOpType

    si, sn = meta["si"], meta["sn"]
    shard_p, nt, gt = meta["shard_p"], meta["nt"], meta["gt"]
    flags = meta["flags"]

    reg_tiles = meta["reg_tiles"]
    reg_rows = reg_tiles * 128
    nreg = (shard_p + reg_rows - 1) // reg_rows
    nc = bacc.Bacc("TRN2", target_bir_lowering=False, debug=False,
                   num_devices=NC, num_swdge_queues=4)

    # ---- I/O ----
    ein = lambda n, s, d=dt.float32: nc.dram_tensor(n, s, d, kind="ExternalInput")
    xT = ein("xT", [17, si])
    xnT = ein("xnT", [9, sn])
    disf_own = ein("disf_own", [128, nt]); disr_own = ein("disr_own", [128, nt])
    invf_own = ein("invf_own", [128, nt]); invr_own = ein("invr_own", [128, nt])
    gidx_f = ein("gidx_f", [128, meta["tot_f"] // 16], dt.int16)
    sidx_f = ein("sidx_f", [128, meta["tot_f"] // 16], dt.int16)
    gidx_r = ein("gidx_r", [128, meta["tot_r"] // 16], dt.int16)
    sidx_r = ein("sidx_r", [128, meta["tot_r"] // 16], dt.int16)
    enc1_Wb = ein("enc1_Wb", [17, 128]); enc2_W = ein("enc2_W", [128, 64])
    enc2_b = ein("enc2_b", [64, 1])
    net1_Wb = ein("net1_Wb", [9, 64]); net2_W = ein("net2_W", [64, 64])
    net2_b = ein("net2_b", [64, 1])
    wcat = ein("wcat", [L, 65, 128]); wcat_root = ein("wcat_root", [L, 65, 128])
    disf_all = ein("disf_all", [128, gt]); disr_all = ein("disr_all", [128, gt])
    ln_g_t = ein("ln_g", [L, 128, 64]); ln_b_t = ein("ln_b", [L, 128, 64])
    out = nc.dram_tensor("out", [shard_p, (L + 1) * D], dt.float32,
                         kind="ExternalOutput")

    # ---- internals ----
    cin_a = nc.dram_tensor("cin_a", [65, shard_p], dt.float32)
    cin_b = nc.dram_tensor("cin_b", [65, shard_p], dt.float32)
    hT_full = nc.dram_tensor("hT_full", [NC, 65, shard_p], dt.float32,
                             addr_space="Shared")
    xcat = nc.dram_tensor("xcat", [NC * shard_p, 128], dt.float32)
    agg_f = [nc.dram_tensor(f"agg_f{r}", [reg_rows + 128, 64], dt.float32)
             for r in range(nreg)]
    agg_r = [nc.dram_tensor(f"agg_r{r}", [reg_rows + 128, 64], dt.float32)
             for r in range(nreg)]

    with tile.TileContext(nc) as tc:
        with (
            tc.tile_pool(name="const", bufs=1) as cpool,
            tc.tile_pool(name="wpool", bufs=2) as wpool,
            tc.tile_pool(name="enc", bufs=3) as epool,
            tc.tile_pool(name="xph", bufs=4) as xpool,
            tc.tile_pool(name="idx", bufs=4) as ipool,
            tc.tile_pool(name="gat", bufs=4) as gpool,
            tc.tile_pool(name="cmb", bufs=10) as mpool,
            tc.tile_pool(name="sml", bufs=3) as spool,
            tc.tile_pool(name="pe", bufs=4, space="PSUM") as pe_pool,
            tc.tile_pool(name="pc", bufs=2, space="PSUM") as pc_pool,
            tc.tile_pool(name="pt", bufs=2, space="PSUM") as pt_pool,
        ):
            # ---------- constants ----------
            disf_sb = cpool.tile([128, gt], dt.float32)
            disr_sb = cpool.tile([128, gt], dt.float32)
            nc.sync.dma_start(out=disf_sb[:], in_=disf_all[:, :])
            nc.sync.dma_start(out=disr_sb[:], in_=disr_all[:, :])
            dfo = cpool.tile([128, nt], dt.float32)
            dro = cpool.tile([128, nt], dt.float32)
            ifo = cpool.tile([128, nt], dt.float32)
            iro = cpool.tile([128, nt], dt.float32)
            nc.sync.dma_start(out=dfo[:], in_=disf_own[:, :])
            nc.sync.dma_start(out=dro[:], in_=disr_own[:, :])
            nc.sync.dma_start(out=ifo[:], in_=invf_own[:, :])
            nc.sync.dma_start(out=iro[:], in_=invr_own[:, :])
            e1w = cpool.tile([17, 128], dt.float32)
            e2w = cpool.tile([128, 64], dt.float32)
            e2b = cpool.tile([64, 1], dt.float32)
            n1w = cpool.tile([9, 64], dt.float32)
            n2w = cpool.tile([64, 64], dt.float32)
            n2b = cpool.tile([64, 1], dt.float32)
            nc.sync.dma_start(out=e1w[:], in_=enc1_Wb[:, :])
            nc.sync.dma_start(out=e2w[:], in_=enc2_W[:, :])
            nc.sync.dma_start(out=e2b[:], in_=enc2_b[:, :])
            nc.sync.dma_start(out=n1w[:], in_=net1_Wb[:, :])
            nc.sync.dma_start(out=n2w[:], in_=net2_W[:, :])
            nc.sync.dma_start(out=n2b[:], in_=net2_b[:, :])
            lng_sb = cpool.tile([128, L * 64], dt.float32)
            lnb_sb = cpool.tile([128, L * 64], dt.float32)
            nc.sync.dma_start(
                out=lng_sb[:].rearrange("p (l d) -> p l d", l=L),
                in_=ln_g_t.ap().rearrange("l p d -> p l d"))
            nc.sync.dma_start(
                out=lnb_sb[:].rearrange("p (l d) -> p l d", l=L),
                in_=ln_b_t.ap().rearrange("l p d -> p l d"))
            onesr = cpool.tile([1, 4096], dt.float32)
            nc.vector.memset(onesr[:], 1.0)
            zeros = cpool.tile([128, 4096], dt.float32)
            nc.vector.memset(zeros[:], 0.0)
            from concourse import masks as _masks
            ident = cpool.tile([128, 128], dt.float32)
            _masks.make_identity(nc, ident[:])

            # ones rows of cin_a / cin_b
            for cin in (cin_a, cin_b):
                for o in range(0, shard_p, 4096):
                    w = min(4096, shard_p - o)
                    nc.sync.dma_start(out=cin[64:65, o:o + w], in_=onesr[:, :w])

            def leaky(dst_ap, src_ap, tmp_tile):
                nc.vector.tensor_scalar(out=tmp_tile, in0=src_ap, scalar1=0.1,
                                        scalar2=None, op0=OP.mult)
                nc.vector.tensor_tensor(out=dst_ap, in0=src_ap, in1=tmp_tile,
                                        op=OP.max)

            # ---------- encoder (own shard, feature-major) ----------
            def encode(inpT, w1, nfeat1, nmid, w2, b2, has_b2, n_nodes, col_base):
                """two-layer MLP in feat-major; writes cin_a[0:64, col_base:...]
                and node-major h0 into out[:, 0:64]."""
                for t0 in range(0, n_nodes, 512):
                    w = min(512, n_nodes - t0)
                    rhs = epool.tile([nfeat1, 512], dt.float32, tag="erhs")
                    nc.sync.dma_start(out=rhs[:, :w], in_=inpT[:, t0:t0 + w])
                    p1 = pe_pool.tile([128, 512], dt.float32, tag="pe")
                    nc.tensor.matmul(p1[:nmid, :w], w1[:], rhs[:nfeat1, :w],
                                     start=True, stop=True)
                    s1 = epool.tile([128, 512], dt.float32, tag="es1")
                    tmp = epool.tile([128, 512], dt.float32, tag="etmp")
                    leaky(s1[:nmid, :w], p1[:nmid, :w], tmp[:nmid, :w])
                    p2 = pe_pool.tile([128, 512], dt.float32, tag="pe")
                    nc.tensor.matmul(p2[:64, :w], w2[:], s1[:nmid, :w],
                                     start=True, stop=True)
                    s2 = epool.tile([64, 512], dt.float32, tag="es2")
                    tmp2 = epool.tile([64, 512], dt.float32, tag="etmp2")
                    if has_b2:
                        badd = epool.tile([64, 512], dt.float32, tag="ebadd")
                        nc.vector.tensor_scalar(out=badd[:, :w], in0=p2[:64, :w],
                                                scalar1=b2[:, 0:1], scalar2=None,
                                                op0=OP.add)
                        leaky(s2[:, :w], badd[:, :w], tmp2[:, :w])
                    else:
                        leaky(s2[:, :w], p2[:64, :w], tmp2[:, :w])
                    nc.sync.dma_start(out=cin_a[0:64, col_base + t0:col_base + t0 + w],
                                      in_=s2[:, :w])
                    # node-major h0 -> out[:, 0:64] via PE transpose
                    for m0 in range(0, w, 128):
                        mw = min(128, w - m0)
                        pt = pt_pool.tile([128, 64], dt.float32, tag="pt")
                        nc.tensor.transpose(pt[:mw, :], s2[:, m0:m0 + mw],
                                            ident[:64, :64])
                        hc = epool.tile([128, 64], dt.float32, tag="ehc")
                        nc.vector.tensor_copy(out=hc[:mw, :], in_=pt[:mw, :])
                        nc.sync.dma_start(
                            out=out[col_base + t0 + m0:col_base + t0 + m0 + mw, 0:64],
                            in_=hc[:mw, :])

            encode(xT, e1w, 17, 128, e2w, e2b, flags["enc2_bias"], si, 0)
            encode(xnT, n1w, 9, 64, n2w, n2b, flags["net2_bias"], sn, si)
            # pad region of cin_a: zero it (avoid NaNs flowing through matmuls)
            padw = shard_p - si - sn
            if padw > 0:
                nc.sync.dma_start(out=cin_a[0:64, si + sn:shard_p],
                                  in_=zeros[0:64, 0:padw])

            # ---------- layers ----------
            cins = [cin_a, cin_b]
            for l in range(L):
                cin_cur = cins[l % 2]
                cin_nxt = cins[(l + 1) % 2]

                nc.gpsimd.collective_compute(
                    "AllGather", OP.bypass,
                    replica_groups=[list(range(NC))],
                    ins=[cin_cur.ap().opt()], outs=[hT_full.ap().opt()])

                wc = wpool.tile([65, 128], dt.float32, tag="wc")
                wcr = wpool.tile([65, 128], dt.float32, tag="wcr")
                nc.sync.dma_start(out=wc[:], in_=wcat[l, :, :])
                nc.sync.dma_start(out=wcr[:], in_=wcat_root[l, :, :])

                # ----- x-phase: xcat = dis * relu(h @ Wcat + b), all shards -----
                for s in range(NC):
                    for g0 in range(0, nt, 4):
                        gn = min(4, nt - g0)   # tiles in this group
                        wdt = gn * 128
                        hT4 = xpool.tile([65, 512], dt.float32, tag="hT4")
                        nc.sync.dma_start(
                            out=hT4[:, :wdt],
                            in_=hT_full[s, :, g0 * 128:g0 * 128 + wdt])
                        px = pe_pool.tile([128, 512], dt.float32, tag="pe")
                        for m in range(gn):
                            nc.tensor.matmul(
                                px[:, m * 128:(m + 1) * 128],
                                hT4[:, m * 128:(m + 1) * 128], wc[:],
                                start=True, stop=True)
                        rl = xpool.tile([128, 512], dt.float32, tag="rl")
                        nc.scalar.activation(out=rl[:, :wdt], in_=px[:, :wdt],
                                             func=AF.Relu)
                        rv = rl[:].rearrange("p (a q) -> p a q", a=4)
                        col = s * nt + g0
                        nc.vector.tensor_tensor(
                            out=rv[:, :gn, 0:64], in0=rv[:, :gn, 0:64],
                            in1=disf_sb[:, col:col + gn].broadcast_to([128, gn, 64]),
                            op=OP.mult)
                        nc.vector.tensor_tensor(
                            out=rv[:, :gn, 64:128], in0=rv[:, :gn, 64:128],
                            in1=disr_sb[:, col:col + gn].broadcast_to([128, gn, 64]),
                            op=OP.mult)
                        r0 = s * shard_p + g0 * 128
                        nc.sync.dma_start(
                            out=xcat[r0:r0 + wdt, :].rearrange(
                                "(a p) d -> p a d", p=128),
                            in_=rv[:, :gn, :])

                # ----- zero agg -----
                for agg in agg_f + agg_r:
                    av = agg.ap().rearrange("(a p) d -> a p d", p=128)
                    for b0 in range(0, reg_tiles, 8):
                        bn = min(8, reg_tiles - b0)
                        nc.sync.dma_start(
                            out=av[b0:b0 + bn].rearrange("a p d -> p a d"),
                            in_=zeros[:, :bn * 64].rearrange(
                                "p (a d) -> p a d", a=bn))

                # ----- edge phase -----
                qn = 0
                for (calls, gi_t, si_t, agg, half) in (
                        (meta["calls_f"], gidx_f, sidx_f, agg_f, 0),
                        (meta["calls_r"], gidx_r, sidx_r, agg_r, 1)):
                    for (off, size, s, rg) in calls:
                        git = ipool.tile([128, CALL_MAX // 16], dt.int16, tag="git")
                        sit = ipool.tile([128, CALL_MAX // 16], dt.int16, tag="sit")
                        nc.sync.dma_start(out=git[:, :size // 16],
                                          in_=gi_t[:, off // 16:(off + size) // 16])
                        nc.sync.dma_start(out=sit[:, :size // 16],
                                          in_=si_t[:, off // 16:(off + size) // 16])
                        gt_ = gpool.tile([128, CALL_MAX // 128, 64], dt.float32,
                                         tag="gt")
                        nc.gpsimd.dma_gather(
                            out_ap=gt_[:, :size // 128, :],
                            in_ap=xcat[s * shard_p:(s + 1) * shard_p,
                                       half * 64:half * 64 + 64],
                            idxs_ap=git[:, :size // 16],
                            num_idxs=size, num_idxs_reg=size,
                            elem_size=64, elem_step=128, queue_num=qn % 4)
                        qn += 1
                        nc.gpsimd.dma_scatter_add(
                            out_ap=agg[rg].ap(),
                            in_ap=gt_[:, :size // 128, :],
                            idxs_ap=sit[:, :size // 16],
                            num_idxs=size, num_idxs_reg=size, elem_size=64,
                            queue_num=qn % 4)
                        qn += 1

                # ----- combine (own nodes) -----
                use_g = flags["ln_g"][l]
                use_b = flags["ln_b"][l]
                for b0 in range(0, nt, 8):
                    bn = min(8, nt - b0)
                    sums = spool.tile([128, 8], dt.float32, tag="sums")
                    sqs = spool.tile([128, 8], dt.float32, tag="sqs")
                    hsums = []
                    for i in range(bn):
                        t = b0 + i
                        cint = mpool.tile([65, 128], dt.float32, tag="cint")
                        nc.sync.dma_start(out=cint[:],
                                          in_=cin_cur[:, t * 128:(t + 1) * 128])
                        p2 = pc_pool.tile([128, 128], dt.float32, tag="p2c")
                        nc.tensor.matmul(p2[:], cint[:], wcr[:],
                                         start=True, stop=True)
                        agf = mpool.tile([128, 64], dt.float32, tag="agf")
                        agr = mpool.tile([128, 64], dt.float32, tag="agr")
                        t_rg, t_lo = t // reg_tiles, t % reg_tiles
                        nc.sync.dma_start(
                            out=agf[:],
                            in_=agg_f[t_rg][t_lo * 128:(t_lo + 1) * 128, :])
                        nc.sync.dma_start(
                            out=agr[:],
                            in_=agg_r[t_rg][t_lo * 128:(t_lo + 1) * 128, :])
                        stf = mpool.tile([128, 64], dt.float32, tag="stf")
                        stv = mpool.tile([128, 64], dt.float32, tag="str")
                        nc.vector.tensor_scalar(
                            out=stf[:], in0=p2[:, 0:64], scalar1=0.0,
                            scalar2=ifo[:, t:t + 1], op0=OP.max, op1=OP.mult)
                        nc.vector.tensor_scalar(
                            out=stv[:], in0=p2[:, 64:128], scalar1=0.0,
                            scalar2=iro[:, t:t + 1], op0=OP.max, op1=OP.mult)
                        af = mpool.tile([128, 64], dt.float32, tag="af")
                        ar = mpool.tile([128, 64], dt.float32, tag="ar")
                        nc.vector.tensor_scalar(
                            out=af[:], in0=agf[:], scalar1=dfo[:, t:t + 1],
                            scalar2=None, op0=OP.mult)
                        nc.vector.tensor_scalar(
                            out=ar[:], in0=agr[:], scalar1=dro[:, t:t + 1],
                            scalar2=None, op0=OP.mult)
                        h1 = mpool.tile([128, 64], dt.float32, tag="h1")
                        h2 = mpool.tile([128, 64], dt.float32, tag="h2")
                        hs = mpool.tile([128, 64], dt.float32, tag="hs")
                        nc.vector.tensor_tensor(out=h1[:], in0=af[:], in1=stf[:],
                                                op=OP.add)
                        nc.vector.tensor_tensor(out=h2[:], in0=ar[:], in1=stv[:],
                                                op=OP.add)
                        nc.vector.tensor_tensor(out=hs[:], in0=h1[:], in1=h2[:],
                                                op=OP.add)
                        sc1 = mpool.tile([128, 64], dt.float32, tag="sc1")
                        nc.scalar.activation(out=sc1[:], in_=hs[:],
                                             func=AF.Identity,
                                             accum_out=sums[:, i:i + 1])
                        sc2 = mpool.tile([128, 64], dt.float32, tag="sc2")
                        nc.scalar.activation(out=sc2[:], in_=hs[:],
                                             func=AF.Square,
                                             accum_out=sqs[:, i:i + 1])
                        hsums.append(hs)
                    # batched stats
                    m8 = spool.tile([128, 8], dt.float32, tag="m8")
                    ex2 = spool.tile([128, 8], dt.float32, tag="ex2")
                    nc.vector.tensor_scalar(out=m8[:, :bn], in0=sums[:, :bn],
                                            scalar1=1.0 / 64, scalar2=None,
                                            op0=OP.mult)
                    nc.vector.tensor_scalar(out=ex2[:, :bn], in0=sqs[:, :bn],
                                            scalar1=1.0 / 64, scalar2=None,
                                            op0=OP.mult)
                    msq = spool.tile([128, 8], dt.float32, tag="msq")
                    nc.vector.tensor_tensor(out=msq[:, :bn], in0=m8[:, :bn],
                                            in1=m8[:, :bn], op=OP.mult)
                    var = spool.tile([128, 8], dt.float32, tag="var")
                    nc.vector.tensor_tensor(out=var[:, :bn], in0=ex2[:, :bn],
                                            in1=msq[:, :bn], op=OP.subtract)
                    vpe = spool.tile([128, 8], dt.float32, tag="vpe")
                    nc.vector.tensor_scalar(out=vpe[:, :bn], in0=var[:, :bn],
                                            scalar1=EPS, scalar2=None, op0=OP.add)
                    sd = spool.tile([128, 8], dt.float32, tag="sd")
                    nc.scalar.activation(out=sd[:, :bn], in_=vpe[:, :bn],
                                         func=AF.Sqrt)
                    rstd = spool.tile([128, 8], dt.float32, tag="rstd")
                    nc.vector.reciprocal(out=rstd[:, :bn], in_=sd[:, :bn])
                    for i in range(bn):
                        t = b0 + i
                        hs = hsums[i]
                        nm = mpool.tile([128, 64], dt.float32, tag="nm")
                        nc.vector.tensor_scalar(
                            out=nm[:], in0=hs[:], scalar1=m8[:, i:i + 1],
                            scalar2=rstd[:, i:i + 1],
                            op0=OP.subtract, op1=OP.mult)
                        cur = nm
                        if use_g:
                            gmul = mpool.tile([128, 64], dt.float32, tag="gmul")
                            nc.vector.tensor_tensor(
                                out=gmul[:], in0=cur[:],
                                in1=lng_sb[:, l * 64:(l + 1) * 64],
                                op=OP.mult)
                            cur = gmul
                        if use_b:
                            badd = mpool.tile([128, 64], dt.float32, tag="lbadd")
                            nc.vector.tensor_tensor(
                                out=badd[:], in0=cur[:],
                                in1=lnb_sb[:, l * 64:(l + 1) * 64],
                                op=OP.add)
                            cur = badd
                        hn = mpool.tile([128, 64], dt.float32, tag="hn")
                        tmp = mpool.tile([128, 64], dt.float32, tag="ltmp")
                        leaky(hn[:], cur[:], tmp[:])
                        nc.sync.dma_start(
                            out=out[t * 128:(t + 1) * 128,
                                    (l + 1) * 64:(l + 2) * 64],
                            in_=hn[:])
                        if l < L - 1:
                            pt = pt_pool.tile([64, 128], dt.float32, tag="pt")
                            nc.tensor.transpose(pt[:], hn[:], ident[:])
                            tp = mpool.tile([64, 128], dt.float32, tag="tp")
                            nc.vector.tensor_copy(out=tp[:], in_=pt[:])
                            nc.sync.dma_start(
                                out=cin_nxt[0:64, t * 128:(t + 1) * 128],
                                in_=tp[:])

    nc.compile()
    return nc


# ---------------------------------------------------------------------------
# entry point
# ---------------------------------------------------------------------------

def kernel(**inputs):
    from concourse.bass_utils import run_bass_kernel_spmd

    meta, per_core = _prep(inputs)
    key = (meta["n_inst"], meta["n_net"], meta["tot_f"], meta["tot_r"],
           tuple(meta["calls_f"]), tuple(meta["calls_r"]),
           tuple(meta["flags"]["ln_g"]), tuple(meta["flags"]["ln_b"]),
           meta["flags"]["enc2_bias"], meta["flags"]["net2_bias"])
    if key not in _CACHE:
        _CACHE.clear()
        _CACHE[key] = _build(meta)
    nc = _CACHE[key]

    res = run_bass_kernel_spmd(nc, per_core, core_ids=list(range(NC)))

    n_inst, n_net = meta["n_inst"], meta["n_net"]
    si, sn, shard_p = meta["si"], meta["sn"], meta["shard_p"]
    outp = np.empty((n_inst + n_net, (L + 1) * D), np.float32)
    for c in range(NC):
        oc = res.results[c]["out"]
        outp[c * si:(c + 1) * si] = oc[:si]
        outp[n_inst + c * sn:n_inst + (c + 1) * sn] = oc[si:si + sn]
    return outp



# revision 11
# speedup vs baseline: 2.2408x; 2.2408x over previous
"""Trainium2 Bass kernel for nn_GNN_node_30279519437414 (GNN message passing).

Self-contained: takes FULL inputs, shards across 8 NeuronCores internally,
returns the FULL output.

Strategy:
  - Nodes sharded across 8 cores; within a core, nodes are spread over 310
    destination tiles of 128 partition slots (~101 nodes/tile) so that the
    per-(tile, source-core) edge groups rarely exceed 128 edges.
  - h is kept resident in SBUF (feat-major, bf16).  Per layer each core
    computes its own x' = dis * relu(h @ Wcat) slice (node-major bf16,
    fwd|rev packed in 128 features = 256B rows), writes it to DRAM packed at
    104 rows/tile (so chunk-local gather indices fit int16), and the x'
    tables are AllGathered.
  - Edge phase: per destination wave of 12 tiles, per source chunk, a single
    dma_gather pulls the source rows of all edges (dest-tile-grouped, padded
    to 128-multiples, group sizes common across cores = max).  Aggregation
    is done on the tensor engine: a one-hot indicator matrix (built on the
    vector engine with is_equal against an iota) is multiplied with the
    gathered messages, accumulating in PSUM per destination tile.  No
    scatter-add, no HBM round trip for the aggregate.
  - Combine (self-term + degree scaling + LayerNorm + leaky) reads the PSUM
    aggregates directly and is fused with the next layer's x' computation.
"""

import sys

sys.path.insert(0, "/opt/trn_rl_repo")

import numpy as np
import ml_dtypes

BF16 = ml_dtypes.bfloat16

NC = 8
D = 64
L = 3
EPS = 1e-5
TI = 248          # instance tiles per core
TN = 62           # net tiles per core
NT = TI + TN      # 310 dest tiles per core
TP = 104          # table rows per tile (packed, 310*104 = 32240 <= int16)
TAB = NT * TP     # 32240 table rows per core
DST = NT * 128    # 39680 dest rows per core
WT = 12           # tiles per wave
NW = (NT + WT - 1) // WT
CALL_MAX = 1024

_CACHE = {}


# ---------------------------------------------------------------------------
# host-side preprocessing
# ---------------------------------------------------------------------------

def _wrap_idx_dram(arr):
    """[S] int16 -> [128, S//16] (16-partition wrap replicated x8)."""
    w = arr.reshape(-1, 16).T.copy()
    return np.ascontiguousarray(np.tile(w, (8, 1)))


def _node_coords(ids, n_inst, si, sn):
    """ref node ids -> (core, tile, pos)."""
    ids = np.asarray(ids, dtype=np.int64)
    is_net = ids >= n_inst
    r = ids - n_inst
    c = np.where(is_net, r // sn, ids // si)
    j = np.where(is_net, r - (r // sn) * sn, ids - (ids // si) * si)
    tile = np.where(is_net, TI + j % TN, j % TI)
    pos = np.where(is_net, j // TN, j // TI)
    return c, tile, pos


def _edge_plan_dir(s_core, s_tab, d_core, d_tile, d_pos):
    """Group edges by (dest core, dest tile, src core); group sizes are the
    max over dest cores, padded to 128.  Returns (gsz [NT,8], S, gidx [8,S]
    int16 chunk-local table rows, dloc [8,S] f32 dest offsets, pads=255)."""
    gid = d_tile * NC + s_core                       # [E]
    cnts = np.zeros((NC, NT * NC), np.int64)
    for c in range(NC):
        cnts[c] = np.bincount(gid[d_core == c], minlength=NT * NC)
    mx = cnts.max(axis=0)
    gsz = ((np.maximum(mx, 1) + 127) // 128) * 128   # [NT*8]

    # group offsets in (wave, chunk, tile) emission order
    w_of_t = np.arange(NT) // WT
    ordk = ((w_of_t[:, None] * NC + np.arange(NC)[None, :]) * NT
            + np.arange(NT)[:, None]).ravel()        # [NT*8] by (t, s)
    order = np.argsort(ordk, kind="stable")
    offs = np.zeros(NT * NC, np.int64)
    offs[order] = np.concatenate([[0], np.cumsum(gsz[order])[:-1]])
    S = int(gsz.sum())

    # per-edge rank within (dest core, group)
    k2 = d_core.astype(np.int64) * (NT * NC) + gid
    o = np.argsort(k2, kind="stable")
    ks = k2[o]
    newg = np.empty(len(ks), dtype=bool)
    newg[0] = True
    np.not_equal(ks[1:], ks[:-1], out=newg[1:])
    starts = np.flatnonzero(newg)
    cnt2 = np.diff(np.r_[starts, len(ks)])
    rank = np.empty(len(ks), np.int64)
    rank[o] = np.arange(len(ks)) - np.repeat(starts, cnt2)
    slot = offs[gid] + rank

    gidx = np.zeros((NC, S), np.int16)
    dloc = np.full((NC, S), 255.0, np.float32)
    gidx[d_core, slot] = s_tab.astype(np.int16)
    dloc[d_core, slot] = d_pos
    return gsz.reshape(NT, NC), S, gidx, dloc


def _prep(inputs):
    n_inst = inputs["x"].shape[0]
    n_net = inputs["x_net"].shape[0]
    si, sn = n_inst // NC, n_net // NC
    N = n_inst + n_net
    assert si <= TI * 128 and sn <= TN * 128
    assert (si + TI - 1) // TI <= TP and (sn + TN - 1) // TN <= TP

    f = lambda k: np.asarray(inputs[k], dtype=np.float32)
    edge_index = inputs["edge_index"]
    row = np.asarray(edge_index[0], dtype=np.int64)
    col = np.asarray(edge_index[1], dtype=np.int64)

    deg_f = (np.bincount(row, minlength=N) + 1).astype(np.float32)
    deg_r = (np.bincount(col, minlength=N) + 1).astype(np.float32)
    dis_f = deg_f ** -0.5
    dis_r = deg_r ** -0.5
    inv_f = (1.0 / deg_f).astype(np.float32)
    inv_r = (1.0 / deg_r).astype(np.float32)

    # per-node dest coordinates for all ref ids
    allc, allt, allp = _node_coords(np.arange(N), n_inst, si, sn)
    drow = allt * 128 + allp                         # dest row within core

    def tabize(a):
        t = np.ones((NC, DST), np.float32)
        t[allc, drow] = a
        return t.reshape(NC, NT, 128).transpose(0, 2, 1).copy()  # [NC,128,NT]

    disf_t = tabize(dis_f)
    disr_t = tabize(dis_r)
    invf_t = tabize(inv_f)
    invr_t = tabize(inv_r)

    # edge plans
    rc, rt, rp = _node_coords(row, n_inst, si, sn)
    cc, ct, cp = _node_coords(col, n_inst, si, sn)
    rtab = (rt * TP + rp).astype(np.int64)           # chunk-local table row
    ctab = (ct * TP + cp).astype(np.int64)
    gsz_f, S_f, gidx_f, dloc_f = _edge_plan_dir(rc, rtab, cc, ct, cp)
    gsz_r, S_r, gidx_r, dloc_r = _edge_plan_dir(cc, ctab, rc, rt, rp)

    # weights
    enc1_Wb = np.vstack([f("enc1_W"), f("enc1_b")[None, :]])      # [17, 128]
    net1_Wb = np.vstack([f("net1_W"), f("net1_b")[None, :]])      # [9, 64]
    enc2_W, enc2_b = f("enc2_W"), f("enc2_b")
    net2_W, net2_b = f("net2_W"), f("net2_b")
    conv_W, conv_b, conv_root = f("conv_W"), f("conv_b"), f("conv_root")
    re_W, re_b, re_root = f("re_W"), f("re_b"), f("re_root")
    ln_g, ln_b = f("ln_g"), f("ln_b")

    wcat = np.zeros((L, 65, 128), np.float32)
    wcat_root = np.zeros((L, 65, 128), np.float32)
    for l in range(L):
        wcat[l, :64, :64] = conv_W[l]
        wcat[l, :64, 64:] = re_W[l]
        wcat[l, 64, :64] = conv_b[l]
        wcat[l, 64, 64:] = re_b[l]
        wcat_root[l] = wcat[l]
        wcat_root[l, 64, :64] += conv_root[l]
        wcat_root[l, 64, 64:] += re_root[l]

    flags = {
        "ln_g": [not np.allclose(ln_g[l], 1.0) for l in range(L)],
        "ln_b": [not np.allclose(ln_b[l], 0.0) for l in range(L)],
    }

    # encoder inputs in table-column order
    x = f("x")
    x_net = f("x_net")
    iota = np.tile(np.arange(128, dtype=np.float32), (128, 8))    # [128,1024]

    per_core = []
    for c in range(NC):
        xT = np.zeros((17, TI * 128), np.float32)
        jj = np.arange(si)
        xT[:16, (jj % TI) * 128 + jj // TI] = x[c * si:(c + 1) * si].T
        xT[16, :] = 1.0
        xnT = np.zeros((9, TN * 128), np.float32)
        jj = np.arange(sn)
        xnT[:8, (jj % TN) * 128 + jj // TN] = x_net[c * sn:(c + 1) * sn].T
        xnT[8, :] = 1.0
        d = {
            "xT": xT.astype(BF16),
            "xnT": xnT.astype(BF16),
            "disf": np.ascontiguousarray(disf_t[c]),
            "disr": np.ascontiguousarray(disr_t[c]),
            "invf": np.ascontiguousarray(invf_t[c]),
            "invr": np.ascontiguousarray(invr_t[c]),
            "gidx_f": _wrap_idx_dram(gidx_f[c]),
            "dloc_f": np.ascontiguousarray(
                dloc_f[c].reshape(-1, 128).T),
            "gidx_r": _wrap_idx_dram(gidx_r[c]),
            "dloc_r": np.ascontiguousarray(
                dloc_r[c].reshape(-1, 128).T),
            # replicated
            "enc1_Wb": enc1_Wb.astype(BF16),
            "enc2_W": np.ascontiguousarray(enc2_W).astype(BF16),
            "enc2_b": enc2_b.reshape(64, 1).copy(),
            "net1_Wb": net1_Wb.astype(BF16),
            "net2_W": np.ascontiguousarray(net2_W).astype(BF16),
            "net2_b": net2_b.reshape(64, 1).copy(),
            "wcat": wcat.astype(BF16),
            "wcat_root": wcat_root.astype(BF16),
            "iota": iota,
            "ln_g": np.ascontiguousarray(
                np.broadcast_to(ln_g[:, None, :], (L, 128, 64))),
            "ln_b": np.ascontiguousarray(
                np.broadcast_to(ln_b[:, None, :], (L, 128, 64))),
        }
        per_core.append(d)

    meta = {
        "n_inst": n_inst, "n_net": n_net, "si": si, "sn": sn,
        "gsz_f": gsz_f, "S_f": S_f, "gsz_r": gsz_r, "S_r": S_r,
        "flags": flags,
    }
    return meta, per_core


# ---------------------------------------------------------------------------
# device program
# ---------------------------------------------------------------------------

def _patch_lane_assignment():
    """Make Tile's DMASW lane choice queue-aware (queue q owns lanes 2q/2q+1)
    so SWDGE-queue round-robin doesn't trip the lane<->queue lock."""
    import concourse.tile_sem_assignment as tsa
    import concourse.mybir as mybir
    import concourse.bass_isa as bass_isa
    if getattr(tsa.TileClockTick, "_q_aware", False):
        return
    orig = tsa.TileClockTick._assign_tick

    def _assign_tick(self, inst):
        if (isinstance(inst, tsa.DMAInst)
                and not isinstance(inst, bass_isa.UserSyncedRemoteDMADescs)
                and inst.engine == mybir.EngineType.Pool
                and self.swdge_sem_count == tsa.NUM_SWDGE_GLOBAL_SEMS):
            qn = getattr(inst, "queue_num", 0) or 0
            if not hasattr(self, "_q_rr"):
                self._q_rr = {}
            r = self._q_rr.get(qn, 0)
            self._q_rr[qn] = r ^ 1
            self.next_sw_dma_idx = (qn * 2 + r) % self.swdge_sem_count
        return orig(self, inst)

    tsa.TileClockTick._assign_tick = _assign_tick
    tsa.TileClockTick._q_aware = True


def _build(meta):
    import concourse.bass as bass
    import concourse.bacc as bacc
    import concourse.mybir as mybir
    from concourse import tile
    from concourse import masks as _masks

    _patch_lane_assignment()

    dt = mybir.dt
    AF = mybir.ActivationFunctionType
    OP = mybir.AluOpType

    gszs = [meta["gsz_f"], meta["gsz_r"]]
    Ss = [meta["S_f"], meta["S_r"]]
    flags = meta["flags"]

    nc = bacc.Bacc("TRN2", target_bir_lowering=False, debug=False,
                   num_devices=NC, num_swdge_queues=4)

    ein = lambda n, s, d=dt.float32: nc.dram_tensor(n, s, d, kind="ExternalInput")
    xT = ein("xT", [17, TI * 128], dt.bfloat16)
    xnT = ein("xnT", [9, TN * 128], dt.bfloat16)
    disf = ein("disf", [128, NT]); disr = ein("disr", [128, NT])
    invf = ein("invf", [128, NT]); invr = ein("invr", [128, NT])
    gidx_d = [ein("gidx_f", [128, Ss[0] // 16], dt.int16),
              ein("gidx_r", [128, Ss[1] // 16], dt.int16)]
    dloc_d = [ein("dloc_f", [128, Ss[0] // 128]),
              ein("dloc_r", [128, Ss[1] // 128])]
    enc1_Wb = ein("enc1_Wb", [17, 128], dt.bfloat16)
    enc2_W = ein("enc2_W", [128, 64], dt.bfloat16)
    enc2_b = ein("enc2_b", [64, 1])
    net1_Wb = ein("net1_Wb", [9, 64], dt.bfloat16)
    net2_W = ein("net2_W", [64, 64], dt.bfloat16)
    net2_b = ein("net2_b", [64, 1])
    wcat_d = ein("wcat", [L, 65, 128], dt.bfloat16)
    wcatr_d = ein("wcat_root", [L, 65, 128], dt.bfloat16)
    iota_d = ein("iota", [128, 1024])
    ln_g_t = ein("ln_g", [L, 128, 64]); ln_b_t = ein("ln_b", [L, 128, 64])
    out = nc.dram_tensor("out", [DST, (L + 1) * D], dt.float32,
                         kind="ExternalOutput")

    xp_own = nc.dram_tensor("xp_own", [TAB, 128], dt.bfloat16)
    xp_full = nc.dram_tensor("xp_full", [NC * TAB, 128], dt.bfloat16,
                             addr_space="Shared")

    with tile.TileContext(nc) as tc:
        with (
            tc.tile_pool(name="const", bufs=1) as cpool,
            tc.tile_pool(name="enc", bufs=3) as epool,
            tc.tile_pool(name="idx", bufs=4) as ipool,
            tc.tile_pool(name="dlc", bufs=4) as dpool,
            tc.tile_pool(name="gat", bufs=4) as gpool,
            tc.tile_pool(name="ind", bufs=4) as npool,
            tc.tile_pool(name="cmb", bufs=4) as mpool,
            tc.tile_pool(name="hst", bufs=WT + 3) as hpool,
            tc.tile_pool(name="sml", bufs=3) as spool,
            tc.tile_pool(name="xps", bufs=3) as xpool,
            tc.tile_pool(name="agg", bufs=2, space="PSUM") as apool,
            tc.tile_pool(name="pmm", bufs=2, space="PSUM") as mmpool,
        ):
            # ---------- constants ----------
            h_res = cpool.tile([65, NT * 128], dt.bfloat16)
            for o in range(0, NT * 128, 4096):
                ww = min(4096, NT * 128 - o)
                nc.vector.memset(h_res[64:65, o:o + ww], 1.0)
            dfo = cpool.tile([128, NT], dt.float32)
            dro = cpool.tile([128, NT], dt.float32)
            ifo = cpool.tile([128, NT], dt.float32)
            iro = cpool.tile([128, NT], dt.float32)
            nc.sync.dma_start(out=dfo[:], in_=disf[:, :])
            nc.sync.dma_start(out=dro[:], in_=disr[:, :])
            nc.sync.dma_start(out=ifo[:], in_=invf[:, :])
            nc.sync.dma_start(out=iro[:], in_=invr[:, :])
            e1w = cpool.tile([17, 128], dt.bfloat16)
            e2w = cpool.tile([128, 64], dt.bfloat16)
            e2b = cpool.tile([64, 1], dt.float32)
            n1w = cpool.tile([9, 64], dt.bfloat16)
            n2w = cpool.tile([64, 64], dt.bfloat16)
            n2b = cpool.tile([64, 1], dt.float32)
            nc.sync.dma_start(out=e1w[:], in_=enc1_Wb[:, :])
            nc.sync.dma_start(out=e2w[:], in_=enc2_W[:, :])
            nc.sync.dma_start(out=e2b[:], in_=enc2_b[:, :])
            nc.sync.dma_start(out=n1w[:], in_=net1_Wb[:, :])
            nc.sync.dma_start(out=n2w[:], in_=net2_W[:, :])
            nc.sync.dma_start(out=n2b[:], in_=net2_b[:, :])
            wc_sb = cpool.tile([65, L * 128], dt.bfloat16)
            wcr_sb = cpool.tile([65, L * 128], dt.bfloat16)
            nc.sync.dma_start(
                out=wc_sb[:].rearrange("p (l d) -> p l d", l=L),
                in_=wcat_d.ap().rearrange("l p d -> p l d"))
            nc.sync.dma_start(
                out=wcr_sb[:].rearrange("p (l d) -> p l d", l=L),
                in_=wcatr_d.ap().rearrange("l p d -> p l d"))
            iota_sb = cpool.tile([128, 1024], dt.float32)
            nc.sync.dma_start(out=iota_sb[:], in_=iota_d[:, :])
            lng_sb = cpool.tile([128, L * 64], dt.float32)
            lnb_sb = cpool.tile([128, L * 64], dt.float32)
            nc.sync.dma_start(
                out=lng_sb[:].rearrange("p (l d) -> p l d", l=L),
                in_=ln_g_t.ap().rearrange("l p d -> p l d"))
            nc.sync.dma_start(
                out=lnb_sb[:].rearrange("p (l d) -> p l d", l=L),
                in_=ln_b_t.ap().rearrange("l p d -> p l d"))
            ident32 = cpool.tile([128, 128], dt.float32)
            _masks.make_identity(nc, ident32[:])
            ident16 = cpool.tile([64, 64], dt.bfloat16)
            _masks.make_identity(nc, ident16[:])

            def leaky(dst_ap, src_ap, tmp_tile):
                nc.vector.tensor_scalar(out=tmp_tile, in0=src_ap, scalar1=0.1,
                                        scalar2=None, op0=OP.mult)
                nc.vector.tensor_tensor(out=dst_ap, in0=src_ap, in1=tmp_tile,
                                        op=OP.max)

            # ---------- x' phase for one tile ----------
            def emit_xphase(l, t):
                px = mmpool.tile([128, 512], dt.float32, tag="mm")
                nc.tensor.matmul(px[:, :128],
                                 h_res[:, t * 128:(t + 1) * 128],
                                 wc_sb[:, l * 128:(l + 1) * 128],
                                 start=True, stop=True)
                xps = xpool.tile([128, 128], dt.bfloat16, tag="xps")
                nc.scalar.activation(out=xps[:, 0:64], in_=px[:, 0:64],
                                     func=AF.Relu, scale=dfo[:, t:t + 1])
                nc.scalar.activation(out=xps[:, 64:128], in_=px[:, 64:128],
                                     func=AF.Relu, scale=dro[:, t:t + 1])
                nc.sync.dma_start(out=xp_own[t * TP:(t + 1) * TP, :],
                                  in_=xps[0:TP, :])

            # ---------- encoder ----------
            def encode(inpT, w1, nf1, nmid, w2, b2, ncols, col_base):
                for t0 in range(0, ncols, 512):
                    w = min(512, ncols - t0)
                    rhs = epool.tile([nf1, 512], dt.bfloat16, tag="erhs")
                    nc.sync.dma_start(out=rhs[:, :w], in_=inpT[:, t0:t0 + w])
                    p1 = mmpool.tile([128, 512], dt.float32, tag="mm")
                    nc.tensor.matmul(p1[:nmid, :w], w1[:], rhs[:nf1, :w],
                                     start=True, stop=True)
                    s1 = epool.tile([128, 512], dt.bfloat16, tag="es1")
                    tmp1 = epool.tile([128, 512], dt.float32, tag="etmp1")
                    leaky(s1[:nmid, :w], p1[:nmid, :w], tmp1[:nmid, :w])
                    p2 = mmpool.tile([128, 512], dt.float32, tag="mm")
                    nc.tensor.matmul(p2[:64, :w], w2[:], s1[:nmid, :w],
                                     start=True, stop=True)
                    s2 = epool.tile([64, 512], dt.bfloat16, tag="es2")
                    badd = epool.tile([64, 512], dt.float32, tag="ebadd")
                    nc.vector.tensor_scalar(out=badd[:, :w], in0=p2[:64, :w],
                                            scalar1=b2[:, 0:1], scalar2=None,
                                            op0=OP.add)
                    tmp2 = epool.tile([64, 512], dt.float32, tag="etmp2")
                    leaky(s2[:, :w], badd[:, :w], tmp2[:, :w])
                    nc.vector.tensor_copy(
                        out=h_res[0:64, col_base + t0:col_base + t0 + w],
                        in_=s2[:, :w])
                    for m0 in range(0, w, 128):
                        mw = min(128, w - m0)
                        pt = mmpool.tile([128, 512], dt.bfloat16, tag="mm")
                        nc.tensor.matmul(pt[:mw, :64], s2[:, m0:m0 + mw],
                                         ident16[:, :], start=True, stop=True,
                                         is_transpose=True)
                        hc = epool.tile([128, 64], dt.float32, tag="ehc")
                        nc.scalar.activation(out=hc[:mw, :], in_=pt[:mw, :64],
                                             func=AF.Copy)
                        nc.sync.dma_start(
                            out=out[col_base + t0 + m0:col_base + t0 + m0 + mw,
                                    0:64],
                            in_=hc[:mw, :])

            encode(xT, e1w, 17, 128, e2w, e2b, TI * 128, 0)
            encode(xnT, n1w, 9, 64, n2w, n2b, TN * 128, TI * 128)

            for t in range(NT):
                emit_xphase(0, t)

            # ---------- layers ----------
            qn = 0
            for l in range(L):
                nc.gpsimd.collective_compute(
                    "AllGather", OP.bypass,
                    replica_groups=[list(range(NC))],
                    ins=[xp_own.ap().opt()], outs=[xp_full.ap().opt()])

                use_g = flags["ln_g"][l]
                use_b = flags["ln_b"][l]
                offs = [0, 0]
                for w in range(NW):
                    tiles = list(range(w * WT, min(NT, (w + 1) * WT)))
                    aggT = [apool.tile([128, 512], dt.float32, tag=f"agg{j}",
                                       name=f"agg{j}")
                            for j in range(3)]

                    def agg_slice(i, d):
                        slot = i * 2 + d
                        return aggT[slot // 8][:, (slot % 8) * 64:
                                               (slot % 8) * 64 + 64]

                    # one PSUM accumulation group per bank (zero regions are
                    # 2KB): first matmul into a bank starts the group (which
                    # marks the whole bank pending-zero), the last stops it.
                    seq = []
                    for dd in range(2):
                        gsz = gszs[dd]
                        for s in range(NC):
                            for i, t in enumerate(tiles):
                                for k2 in range(int(gsz[t, s]) // 128):
                                    seq.append(((i * 2 + dd) // 8,
                                                (dd, s, i, k2)))
                    first_mm = {}
                    last_mm = {}
                    for j, kk in seq:
                        if j not in first_mm:
                            first_mm[j] = kk
                        last_mm[j] = kk
                    first_mm = set(first_mm.values())
                    last_mm = set(last_mm.values())

                    for dd in range(2):
                        gsz = gszs[dd]
                        for s in range(NC):
                            u0 = offs[dd]
                            usz = int(gsz[tiles[0]:tiles[-1] + 1, s].sum())
                            dl = dpool.tile([128, 64], dt.float32, tag="dloc")
                            nc.sync.dma_start(
                                out=dl[:, :usz // 128],
                                in_=dloc_d[dd][:, u0 // 128:(u0 + usz) // 128])
                            # gather calls
                            gts = []
                            k = 0
                            while k < usz:
                                csz = min(CALL_MAX, usz - k)
                                git = ipool.tile([128, CALL_MAX // 16],
                                                 dt.int16, tag="git")
                                nc.sync.dma_start(
                                    out=git[:, :csz // 16],
                                    in_=gidx_d[dd][:, (u0 + k) // 16:
                                                   (u0 + k + csz) // 16])
                                gt = gpool.tile([128, CALL_MAX // 128, 128],
                                                dt.bfloat16, tag="gat")
                                nc.gpsimd.dma_gather(
                                    out_ap=gt[:, :csz // 128, :],
                                    in_ap=xp_full[s * TAB:(s + 1) * TAB, :],
                                    idxs_ap=git[:, :csz // 16],
                                    num_idxs=csz, num_idxs_reg=csz,
                                    elem_size=128, elem_step=128,
                                    queue_num=qn % 4)
                                qn += 1
                                gts.append((k, csz, gt))
                                k += csz
                            # indicators per 1024-slot block
                            inds = []
                            b = 0
                            while b < usz:
                                bsz = min(1024, usz - b)
                                ind = npool.tile([128, 8, 128], dt.bfloat16,
                                                 tag="ind")
                                nc.vector.tensor_tensor(
                                    out=ind[:, :bsz // 128, :],
                                    in0=iota_sb[:, :bsz].rearrange(
                                        "p (a j) -> p a j", j=128),
                                    in1=dl[:, b // 128:(b + bsz) // 128]
                                        .broadcast_to([128, bsz // 128, 128]),
                                    op=OP.is_equal)
                                inds.append(ind)
                                b += 1024
                            # matmuls
                            rel = 0
                            for i, t in enumerate(tiles):
                                n128 = int(gsz[t, s]) // 128
                                for k2 in range(n128):
                                    r = rel + k2 * 128
                                    ci = r // CALL_MAX
                                    k0, csz0, gt0 = gts[ci]
                                    kk = (dd, s, i, k2)
                                    nc.tensor.matmul(
                                        agg_slice(i, dd),
                                        inds[r // 1024][:, (r % 1024) // 128, :],
                                        gt0[:, (r - k0) // 128,
                                            dd * 64:dd * 64 + 64],
                                        start=(kk in first_mm),
                                        stop=(kk in last_mm))
                                rel += n128 * 128
                            offs[dd] += usz

                    # ----- combine wave -----
                    nwt = len(tiles)
                    sums = spool.tile([128, WT], dt.float32, tag="sums")
                    sqs = spool.tile([128, WT], dt.float32, tag="sqs")
                    hss = []
                    for i, t in enumerate(tiles):
                        p2 = mmpool.tile([128, 512], dt.float32, tag="mm")
                        nc.tensor.matmul(p2[:, :128],
                                         h_res[:, t * 128:(t + 1) * 128],
                                         wcr_sb[:, l * 128:(l + 1) * 128],
                                         start=True, stop=True)
                        stf = mpool.tile([128, 64], dt.float32, tag="stf")
                        stv = mpool.tile([128, 64], dt.float32, tag="stv")
                        nc.scalar.activation(out=stf[:], in_=p2[:, 0:64],
                                             func=AF.Relu,
                                             scale=ifo[:, t:t + 1])
                        nc.scalar.activation(out=stv[:], in_=p2[:, 64:128],
                                             func=AF.Relu,
                                             scale=iro[:, t:t + 1])
                        af = mpool.tile([128, 64], dt.float32, tag="af")
                        ar = mpool.tile([128, 64], dt.float32, tag="ar")
                        nc.vector.tensor_scalar(
                            out=af[:], in0=agg_slice(i, 0),
                            scalar1=dfo[:, t:t + 1], scalar2=None,
                            op0=OP.mult)
                        nc.vector.tensor_scalar(
                            out=ar[:], in0=agg_slice(i, 1),
                            scalar1=dro[:, t:t + 1], scalar2=None,
                            op0=OP.mult)
                        h1 = mpool.tile([128, 64], dt.float32, tag="h1")
                        h2 = mpool.tile([128, 64], dt.float32, tag="h2")
                        hs = hpool.tile([128, 64], dt.float32, tag="hs")
                        nc.vector.tensor_tensor(out=h1[:], in0=af[:],
                                                in1=stf[:], op=OP.add)
                        nc.vector.tensor_tensor(out=h2[:], in0=ar[:],
                                                in1=stv[:], op=OP.add)
                        nc.vector.tensor_tensor(out=hs[:], in0=h1[:],
                                                in1=h2[:], op=OP.add)
                        sc1 = mpool.tile([128, 64], dt.float32, tag="sc1")
                        sc2 = mpool.tile([128, 64], dt.float32, tag="sc2")
                        nc.scalar.activation(out=sc1[:], in_=hs[:],
                                             func=AF.Identity,
                                             accum_out=sums[:, i:i + 1])
                        nc.scalar.activation(out=sc2[:], in_=hs[:],
                                             func=AF.Square,
                                             accum_out=sqs[:, i:i + 1])
                        hss.append(hs)
                    m8 = spool.tile([128, WT], dt.float32, tag="m8")
                    ex2 = spool.tile([128, WT], dt.float32, tag="ex2")
                    nc.vector.tensor_scalar(out=m8[:, :nwt], in0=sums[:, :nwt],
                                            scalar1=1.0 / 64, scalar2=None,
                                            op0=OP.mult)
                    nc.vector.tensor_scalar(out=ex2[:, :nwt], in0=sqs[:, :nwt],
                                            scalar1=1.0 / 64, scalar2=None,
                                            op0=OP.mult)
                    msq = spool.tile([128, WT], dt.float32, tag="msq")
                    nc.vector.tensor_tensor(out=msq[:, :nwt], in0=m8[:, :nwt],
                                            in1=m8[:, :nwt], op=OP.mult)
                    var = spool.tile([128, WT], dt.float32, tag="var")
                    nc.vector.tensor_tensor(out=var[:, :nwt], in0=ex2[:, :nwt],
                                            in1=msq[:, :nwt], op=OP.subtract)
                    vpe = spool.tile([128, WT], dt.float32, tag="vpe")
                    nc.vector.tensor_scalar(out=vpe[:, :nwt], in0=var[:, :nwt],
                                            scalar1=EPS, scalar2=None,
                                            op0=OP.add)
                    sd = spool.tile([128, WT], dt.float32, tag="sd")
                    nc.scalar.activation(out=sd[:, :nwt], in_=vpe[:, :nwt],
                                         func=AF.Sqrt)
                    rstd = spool.tile([128, WT], dt.float32, tag="rstd")
                    nc.vector.reciprocal(out=rstd[:, :nwt], in_=sd[:, :nwt])
                    for i, t in enumerate(tiles):
                        hs = hss[i]
                        nm = mpool.tile([128, 64], dt.float32, tag="nm")
                        nc.vector.tensor_scalar(
                            out=nm[:], in0=hs[:], scalar1=m8[:, i:i + 1],
                            scalar2=rstd[:, i:i + 1],
                            op0=OP.subtract, op1=OP.mult)
                        cur = nm
                        if use_g:
                            gm = mpool.tile([128, 64], dt.float32, tag="gm")
                            nc.vector.tensor_tensor(
                                out=gm[:], in0=cur[:],
                                in1=lng_sb[:, l * 64:(l + 1) * 64], op=OP.mult)
                            cur = gm
                        if use_b:
                            bm = mpool.tile([128, 64], dt.float32, tag="bm")
                            nc.vector.tensor_tensor(
                                out=bm[:], in0=cur[:],
                                in1=lnb_sb[:, l * 64:(l + 1) * 64], op=OP.add)
                            cur = bm
                        hn = mpool.tile([128, 64], dt.float32, tag="hn")
                        ltmp = mpool.tile([128, 64], dt.float32, tag="ltmp")
                        leaky(hn[:], cur[:], ltmp[:])
                        nc.sync.dma_start(
                            out=out[t * 128:(t + 1) * 128,
                                    (l + 1) * 64:(l + 2) * 64],
                            in_=hn[:])
                        if l < L - 1:
                            pt = mmpool.tile([128, 512], dt.float32, tag="mm")
                            nc.tensor.matmul(pt[:64, :128], hn[:],
                                             ident32[:, :], start=True,
                                             stop=True, is_transpose=True)
                            nc.scalar.activation(
                                out=h_res[0:64, t * 128:(t + 1) * 128],
                                in_=pt[:64, :128], func=AF.Copy)
                            emit_xphase(l + 1, t)

    nc.compile()
    return nc


# ---------------------------------------------------------------------------
# entry point
# ---------------------------------------------------------------------------

def kernel(**inputs):
    from concourse.bass_utils import run_bass_kernel_spmd

    meta, per_core = _prep(inputs)
    key = (meta["n_inst"], meta["n_net"], meta["S_f"], meta["S_r"],
           meta["gsz_f"].tobytes(), meta["gsz_r"].tobytes(),
           tuple(meta["flags"]["ln_g"]), tuple(meta["flags"]["ln_b"]))
    if key not in _CACHE:
        _CACHE.clear()
        _CACHE[key] = _build(meta)
    nc = _CACHE[key]

    res = run_bass_kernel_spmd(nc, per_core, core_ids=list(range(NC)))

    n_inst, n_net = meta["n_inst"], meta["n_net"]
    si, sn = meta["si"], meta["sn"]
    outp = np.empty((n_inst + n_net, (L + 1) * D), np.float32)
    ji = np.arange(si)
    ri = (ji % TI) * 128 + ji // TI
    jn = np.arange(sn)
    rn = (TI + jn % TN) * 128 + jn // TN
    for c in range(NC):
        oc = res.results[c]["out"]
        outp[c * si:(c + 1) * si] = oc[ri]
        outp[n_inst + c * sn:n_inst + (c + 1) * sn] = oc[rn]
    return outp


# revision 19
# speedup vs baseline: 2.7955x; 1.2475x over previous
"""Trainium2 Bass kernel for nn_GNN_node_30279519437414 (GNN message passing).

Self-contained: takes FULL inputs, shards across 8 NeuronCores internally,
returns the FULL output.

Strategy:
  - Nodes sharded across 8 cores; within a core, nodes are spread over 310
    destination tiles of 128 partition slots (~101 nodes/tile) so that the
    per-(tile, source-core) edge groups rarely exceed 128 edges.
  - h is kept resident in SBUF (feat-major, bf16).  Per layer each core
    computes its own x' = dis * relu(h @ Wcat) slice (node-major bf16,
    fwd|rev packed in 128 features = 256B rows), writes it to DRAM packed at
    104 rows/tile (so chunk-local gather indices fit int16), and the x'
    tables are AllGathered.
  - Edge phase: per destination wave of 12 tiles, per source chunk, a single
    dma_gather pulls the source rows of all edges (dest-tile-grouped, padded
    to 128-multiples, group sizes common across cores = max).  Aggregation
    is done on the tensor engine: a one-hot indicator matrix (built on the
    vector engine with is_equal against an iota) is multiplied with the
    gathered messages, accumulating in PSUM per destination tile.  No
    scatter-add, no HBM round trip for the aggregate.
  - Combine (self-term + degree scaling + LayerNorm + leaky) reads the PSUM
    aggregates directly and is fused with the next layer's x' computation.
"""

import sys

sys.path.insert(0, "/opt/trn_rl_repo")

import numpy as np
import ml_dtypes

BF16 = ml_dtypes.bfloat16

NC = 8
D = 64
L = 3
EPS = 1e-5
TI = 248          # instance tiles per core
TN = 62           # net tiles per core
NT = TI + TN      # 310 dest tiles per core
TP = 104          # table rows per tile (packed, 310*104 = 32240 <= int16)
TAB = NT * TP     # 32240 table rows per core
DST = NT * 128    # 39680 dest rows per core
WT = 12           # tiles per wave
NW = (NT + WT - 1) // WT
CALL_MAX = 1024

_CACHE = {}


# ---------------------------------------------------------------------------
# host-side preprocessing
# ---------------------------------------------------------------------------

def _wrap_idx_dram(arr):
    """[S] int16 -> [128, S//16] (16-partition wrap replicated x8)."""
    w = arr.reshape(-1, 16).T.copy()
    return np.ascontiguousarray(np.tile(w, (8, 1)))


def _node_coords(ids, n_inst, si, sn):
    """ref node ids -> (core, tile, pos)."""
    ids = np.asarray(ids, dtype=np.int64)
    is_net = ids >= n_inst
    r = ids - n_inst
    c = np.where(is_net, r // sn, ids // si)
    j = np.where(is_net, r - (r // sn) * sn, ids - (ids // si) * si)
    tile = np.where(is_net, TI + j % TN, j % TI)
    pos = np.where(is_net, j // TN, j // TI)
    return c, tile, pos


def _edge_plan_dir(s_core, s_tab, d_core, d_tile, d_pos):
    """Group edges by (dest core, dest tile, src core); group sizes are the
    max over dest cores, padded to 128.  Returns (gsz [NT,8], S, gidx [8,S]
    int16 chunk-local table rows, dloc [8,S] f32 dest offsets, pads=255)."""
    gid = d_tile * NC + s_core                       # [E]
    cnts = np.zeros((NC, NT * NC), np.int64)
    for c in range(NC):
        cnts[c] = np.bincount(gid[d_core == c], minlength=NT * NC)
    mx = cnts.max(axis=0)
    gsz = ((np.maximum(mx, 1) + 127) // 128) * 128   # [NT*8]

    # group offsets in (wave, chunk, tile) emission order
    w_of_t = np.arange(NT) // WT
    ordk = ((w_of_t[:, None] * NC + np.arange(NC)[None, :]) * NT
            + np.arange(NT)[:, None]).ravel()        # [NT*8] by (t, s)
    order = np.argsort(ordk, kind="stable")
    offs = np.zeros(NT * NC, np.int64)
    offs[order] = np.concatenate([[0], np.cumsum(gsz[order])[:-1]])
    S = int(gsz.sum())

    # per-edge rank within (dest core, group)
    k2 = d_core.astype(np.int64) * (NT * NC) + gid
    o = np.argsort(k2, kind="stable")
    ks = k2[o]
    newg = np.empty(len(ks), dtype=bool)
    newg[0] = True
    np.not_equal(ks[1:], ks[:-1], out=newg[1:])
    starts = np.flatnonzero(newg)
    cnt2 = np.diff(np.r_[starts, len(ks)])
    rank = np.empty(len(ks), np.int64)
    rank[o] = np.arange(len(ks)) - np.repeat(starts, cnt2)
    slot = offs[gid] + rank

    gidx = np.zeros((NC, S), np.int16)
    dloc = np.full((NC, S), 255.0, np.float32)
    gidx[d_core, slot] = s_tab.astype(np.int16)
    dloc[d_core, slot] = d_pos
    return gsz.reshape(NT, NC), S, gidx, dloc


def _prep(inputs):
    n_inst = inputs["x"].shape[0]
    n_net = inputs["x_net"].shape[0]
    si, sn = n_inst // NC, n_net // NC
    N = n_inst + n_net
    assert si <= TI * 128 and sn <= TN * 128
    assert (si + TI - 1) // TI <= TP and (sn + TN - 1) // TN <= TP

    f = lambda k: np.asarray(inputs[k], dtype=np.float32)
    edge_index = inputs["edge_index"]
    row = np.asarray(edge_index[0], dtype=np.int64)
    col = np.asarray(edge_index[1], dtype=np.int64)

    deg_f = (np.bincount(row, minlength=N) + 1).astype(np.float32)
    deg_r = (np.bincount(col, minlength=N) + 1).astype(np.float32)
    dis_f = deg_f ** -0.5
    dis_r = deg_r ** -0.5
    inv_f = (1.0 / deg_f).astype(np.float32)
    inv_r = (1.0 / deg_r).astype(np.float32)

    # per-node dest coordinates for all ref ids
    allc, allt, allp = _node_coords(np.arange(N), n_inst, si, sn)
    drow = allt * 128 + allp                         # dest row within core

    def tabize(a):
        t = np.ones((NC, DST), np.float32)
        t[allc, drow] = a
        return t.reshape(NC, NT, 128).transpose(0, 2, 1).copy()  # [NC,128,NT]

    disf_t = tabize(dis_f)
    disr_t = tabize(dis_r)
    invf_t = tabize(inv_f)
    invr_t = tabize(inv_r)

    # edge plans
    rc, rt, rp = _node_coords(row, n_inst, si, sn)
    cc, ct, cp = _node_coords(col, n_inst, si, sn)
    rtab = (rt * TP + rp).astype(np.int64)           # chunk-local table row
    ctab = (ct * TP + cp).astype(np.int64)
    gsz_f, S_f, gidx_f, dloc_f = _edge_plan_dir(rc, rtab, cc, ct, cp)
    gsz_r, S_r, gidx_r, dloc_r = _edge_plan_dir(cc, ctab, rc, rt, rp)

    # weights
    enc1_Wb = np.vstack([f("enc1_W"), f("enc1_b")[None, :]])      # [17, 128]
    net1_Wb = np.vstack([f("net1_W"), f("net1_b")[None, :]])      # [9, 64]
    enc2_W, enc2_b = f("enc2_W"), f("enc2_b")
    net2_W, net2_b = f("net2_W"), f("net2_b")
    conv_W, conv_b, conv_root = f("conv_W"), f("conv_b"), f("conv_root")
    re_W, re_b, re_root = f("re_W"), f("re_b"), f("re_root")
    ln_g, ln_b = f("ln_g"), f("ln_b")

    wcat = np.zeros((L, 65, 128), np.float32)
    wcat_root = np.zeros((L, 65, 128), np.float32)
    for l in range(L):
        wcat[l, :64, :64] = conv_W[l]
        wcat[l, :64, 64:] = re_W[l]
        wcat[l, 64, :64] = conv_b[l]
        wcat[l, 64, 64:] = re_b[l]
        wcat_root[l] = wcat[l]
        wcat_root[l, 64, :64] += conv_root[l]
        wcat_root[l, 64, 64:] += re_root[l]

    flags = {
        "ln_g": [not np.allclose(ln_g[l], 1.0) for l in range(L)],
        "ln_b": [not np.allclose(ln_b[l], 0.0) for l in range(L)],
    }

    # encoder inputs in table-column order
    x = f("x")
    x_net = f("x_net")
    iota = np.tile(np.arange(128, dtype=np.float32), (128, 8))    # [128,1024]

    per_core = []
    for c in range(NC):
        xT = np.zeros((17, TI * 128), np.float32)
        jj = np.arange(si)
        xT[:16, (jj % TI) * 128 + jj // TI] = x[c * si:(c + 1) * si].T
        xT[16, :] = 1.0
        xnT = np.zeros((9, TN * 128), np.float32)
        jj = np.arange(sn)
        xnT[:8, (jj % TN) * 128 + jj // TN] = x_net[c * sn:(c + 1) * sn].T
        xnT[8, :] = 1.0
        d = {
            "xT": xT.astype(BF16),
            "xnT": xnT.astype(BF16),
            "disf": np.ascontiguousarray(disf_t[c]),
            "disr": np.ascontiguousarray(disr_t[c]),
            "invf": np.ascontiguousarray(invf_t[c]),
            "invr": np.ascontiguousarray(invr_t[c]),
            "gidx_f": _wrap_idx_dram(gidx_f[c]),
            "dloc_f": np.ascontiguousarray(
                dloc_f[c].reshape(-1, 128).T).astype(BF16),
            "gidx_r": _wrap_idx_dram(gidx_r[c]),
            "dloc_r": np.ascontiguousarray(
                dloc_r[c].reshape(-1, 128).T).astype(BF16),
            # replicated
            "enc1_Wb": enc1_Wb.astype(BF16),
            "enc2_W": np.ascontiguousarray(enc2_W).astype(BF16),
            "enc2_b": enc2_b.reshape(64, 1).copy(),
            "net1_Wb": net1_Wb.astype(BF16),
            "net2_W": np.ascontiguousarray(net2_W).astype(BF16),
            "net2_b": net2_b.reshape(64, 1).copy(),
            "wcat": wcat.astype(BF16),
            "wcat_root": wcat_root.astype(BF16),
            "iota": iota.astype(BF16),
            "ln_g": np.ascontiguousarray(
                np.broadcast_to(ln_g[:, None, :], (L, 128, 64))),
            "ln_b": np.ascontiguousarray(
                np.broadcast_to(ln_b[:, None, :], (L, 128, 64))),
        }
        per_core.append(d)

    meta = {
        "n_inst": n_inst, "n_net": n_net, "si": si, "sn": sn,
        "gsz_f": gsz_f, "S_f": S_f, "gsz_r": gsz_r, "S_r": S_r,
        "flags": flags,
    }
    return meta, per_core


# ---------------------------------------------------------------------------
# device program
# ---------------------------------------------------------------------------

def _patch_lane_assignment():
    """Make Tile's DMASW lane choice queue-aware (queue q owns lanes 2q/2q+1)
    so SWDGE-queue round-robin doesn't trip the lane<->queue lock."""
    import concourse.tile_sem_assignment as tsa
    import concourse.mybir as mybir
    import concourse.bass_isa as bass_isa
    if getattr(tsa.TileClockTick, "_q_aware", False):
        return
    orig = tsa.TileClockTick._assign_tick

    def _assign_tick(self, inst):
        if (isinstance(inst, tsa.DMAInst)
                and not isinstance(inst, bass_isa.UserSyncedRemoteDMADescs)
                and inst.engine == mybir.EngineType.Pool
                and self.swdge_sem_count == tsa.NUM_SWDGE_GLOBAL_SEMS):
            qn = getattr(inst, "queue_num", 0) or 0
            if not hasattr(self, "_q_rr"):
                self._q_rr = {}
            r = self._q_rr.get(qn, 0)
            self._q_rr[qn] = r ^ 1
            self.next_sw_dma_idx = (qn * 2 + r) % self.swdge_sem_count
        return orig(self, inst)

    tsa.TileClockTick._assign_tick = _assign_tick
    tsa.TileClockTick._q_aware = True


def _build(meta):
    import concourse.bass as bass
    import concourse.bacc as bacc
    import concourse.mybir as mybir
    from concourse import tile
    from concourse import masks as _masks

    _patch_lane_assignment()

    dt = mybir.dt
    AF = mybir.ActivationFunctionType
    OP = mybir.AluOpType

    gszs = [meta["gsz_f"], meta["gsz_r"]]
    Ss = [meta["S_f"], meta["S_r"]]
    flags = meta["flags"]

    nc = bacc.Bacc("TRN2", target_bir_lowering=False, debug=False,
                   num_devices=NC, num_swdge_queues=4)

    ein = lambda n, s, d=dt.float32: nc.dram_tensor(n, s, d, kind="ExternalInput")
    xT = ein("xT", [17, TI * 128], dt.bfloat16)
    xnT = ein("xnT", [9, TN * 128], dt.bfloat16)
    disf = ein("disf", [128, NT]); disr = ein("disr", [128, NT])
    invf = ein("invf", [128, NT]); invr = ein("invr", [128, NT])
    gidx_d = [ein("gidx_f", [128, Ss[0] // 16], dt.int16),
              ein("gidx_r", [128, Ss[1] // 16], dt.int16)]
    dloc_d = [ein("dloc_f", [128, Ss[0] // 128], dt.bfloat16),
              ein("dloc_r", [128, Ss[1] // 128], dt.bfloat16)]
    enc1_Wb = ein("enc1_Wb", [17, 128], dt.bfloat16)
    enc2_W = ein("enc2_W", [128, 64], dt.bfloat16)
    enc2_b = ein("enc2_b", [64, 1])
    net1_Wb = ein("net1_Wb", [9, 64], dt.bfloat16)
    net2_W = ein("net2_W", [64, 64], dt.bfloat16)
    net2_b = ein("net2_b", [64, 1])
    wcat_d = ein("wcat", [L, 65, 128], dt.bfloat16)
    wcatr_d = ein("wcat_root", [L, 65, 128], dt.bfloat16)
    iota_d = ein("iota", [128, 1024], dt.bfloat16)
    ln_g_t = ein("ln_g", [L, 128, 64]); ln_b_t = ein("ln_b", [L, 128, 64])
    out = nc.dram_tensor("out", [DST, (L + 1) * D], dt.float32,
                         kind="ExternalOutput")

    xp_own = nc.dram_tensor("xp_own", [TAB, 128], dt.bfloat16)
    xp_full = nc.dram_tensor("xp_full", [NC * TAB, 128], dt.bfloat16,
                             addr_space="Shared")

    with tile.TileContext(nc) as tc:
        with (
            tc.tile_pool(name="const", bufs=1) as cpool,
            tc.tile_pool(name="enc", bufs=3) as epool,
            tc.tile_pool(name="idx", bufs=4) as ipool,
            tc.tile_pool(name="dlc", bufs=4) as dpool,
            tc.tile_pool(name="gat", bufs=6) as gpool,
            tc.tile_pool(name="ind", bufs=6) as npool,
            tc.tile_pool(name="cmb", bufs=4) as mpool,
            tc.tile_pool(name="hst", bufs=WT + 3) as hpool,
            tc.tile_pool(name="sml", bufs=3) as spool,
            tc.tile_pool(name="xps", bufs=3) as xpool,
            tc.tile_pool(name="agg", bufs=2, space="PSUM") as apool,
            tc.tile_pool(name="pmm", bufs=2, space="PSUM") as mmpool,
        ):
            # ---------- constants ----------
            h_res = cpool.tile([65, NT * 128], dt.bfloat16)
            for o in range(0, NT * 128, 4096):
                ww = min(4096, NT * 128 - o)
                nc.vector.memset(h_res[64:65, o:o + ww], 1.0)
            dfo = cpool.tile([128, NT], dt.float32)
            dro = cpool.tile([128, NT], dt.float32)
            ifo = cpool.tile([128, NT], dt.float32)
            iro = cpool.tile([128, NT], dt.float32)
            nc.sync.dma_start(out=dfo[:], in_=disf[:, :])
            nc.sync.dma_start(out=dro[:], in_=disr[:, :])
            nc.sync.dma_start(out=ifo[:], in_=invf[:, :])
            nc.sync.dma_start(out=iro[:], in_=invr[:, :])
            e1w = cpool.tile([17, 128], dt.bfloat16)
            e2w = cpool.tile([128, 64], dt.bfloat16)
            e2b = cpool.tile([64, 1], dt.float32)
            n1w = cpool.tile([9, 64], dt.bfloat16)
            n2w = cpool.tile([64, 64], dt.bfloat16)
            n2b = cpool.tile([64, 1], dt.float32)
            nc.sync.dma_start(out=e1w[:], in_=enc1_Wb[:, :])
            nc.sync.dma_start(out=e2w[:], in_=enc2_W[:, :])
            nc.sync.dma_start(out=e2b[:], in_=enc2_b[:, :])
            nc.sync.dma_start(out=n1w[:], in_=net1_Wb[:, :])
            nc.sync.dma_start(out=n2w[:], in_=net2_W[:, :])
            nc.sync.dma_start(out=n2b[:], in_=net2_b[:, :])
            wc_sb = cpool.tile([65, L * 128], dt.bfloat16)
            wcr_sb = cpool.tile([65, L * 128], dt.bfloat16)
            nc.sync.dma_start(
                out=wc_sb[:].rearrange("p (l d) -> p l d", l=L),
                in_=wcat_d.ap().rearrange("l p d -> p l d"))
            nc.sync.dma_start(
                out=wcr_sb[:].rearrange("p (l d) -> p l d", l=L),
                in_=wcatr_d.ap().rearrange("l p d -> p l d"))
            iota_sb = cpool.tile([128, 1024], dt.bfloat16)
            nc.sync.dma_start(out=iota_sb[:], in_=iota_d[:, :])
            lng_sb = cpool.tile([128, L * 64], dt.float32)
            lnb_sb = cpool.tile([128, L * 64], dt.float32)
            nc.sync.dma_start(
                out=lng_sb[:].rearrange("p (l d) -> p l d", l=L),
                in_=ln_g_t.ap().rearrange("l p d -> p l d"))
            nc.sync.dma_start(
                out=lnb_sb[:].rearrange("p (l d) -> p l d", l=L),
                in_=ln_b_t.ap().rearrange("l p d -> p l d"))
            ident32 = cpool.tile([128, 128], dt.float32)
            _masks.make_identity(nc, ident32[:])
            ident16 = cpool.tile([64, 64], dt.bfloat16)
            _masks.make_identity(nc, ident16[:])

            def leaky(dst_ap, src_ap, tmp_tile):
                nc.vector.tensor_scalar(out=tmp_tile, in0=src_ap, scalar1=0.1,
                                        scalar2=None, op0=OP.mult)
                nc.vector.tensor_tensor(out=dst_ap, in0=src_ap, in1=tmp_tile,
                                        op=OP.max)

            # ---------- x' phase for one tile ----------
            def emit_xphase(l, t):
                px = mmpool.tile([128, 512], dt.float32, tag="mm")
                nc.tensor.matmul(px[:, :128],
                                 h_res[:, t * 128:(t + 1) * 128],
                                 wc_sb[:, l * 128:(l + 1) * 128],
                                 start=True, stop=True)
                xps = xpool.tile([128, 128], dt.bfloat16, tag="xps")
                nc.scalar.activation(out=xps[:, 0:64], in_=px[:, 0:64],
                                     func=AF.Relu, scale=dfo[:, t:t + 1])
                nc.scalar.activation(out=xps[:, 64:128], in_=px[:, 64:128],
                                     func=AF.Relu, scale=dro[:, t:t + 1])
                nc.sync.dma_start(out=xp_own[t * TP:(t + 1) * TP, :],
                                  in_=xps[0:TP, :])

            # ---------- encoder ----------
            def encode(inpT, w1, nf1, nmid, w2, b2, ncols, col_base):
                for t0 in range(0, ncols, 512):
                    w = min(512, ncols - t0)
                    rhs = epool.tile([nf1, 512], dt.bfloat16, tag="erhs")
                    nc.sync.dma_start(out=rhs[:, :w], in_=inpT[:, t0:t0 + w])
                    p1 = mmpool.tile([128, 512], dt.float32, tag="mm")
                    nc.tensor.matmul(p1[:nmid, :w], w1[:], rhs[:nf1, :w],
                                     start=True, stop=True)
                    s1 = epool.tile([128, 512], dt.bfloat16, tag="es1")
                    tmp1 = epool.tile([128, 512], dt.float32, tag="etmp1")
                    leaky(s1[:nmid, :w], p1[:nmid, :w], tmp1[:nmid, :w])
                    p2 = mmpool.tile([128, 512], dt.float32, tag="mm")
                    nc.tensor.matmul(p2[:64, :w], w2[:], s1[:nmid, :w],
                                     start=True, stop=True)
                    s2 = epool.tile([64, 512], dt.bfloat16, tag="es2")
                    badd = epool.tile([64, 512], dt.float32, tag="ebadd")
                    nc.vector.tensor_scalar(out=badd[:, :w], in0=p2[:64, :w],
                                            scalar1=b2[:, 0:1], scalar2=None,
                                            op0=OP.add)
                    tmp2 = epool.tile([64, 512], dt.float32, tag="etmp2")
                    leaky(s2[:, :w], badd[:, :w], tmp2[:, :w])
                    nc.vector.tensor_copy(
                        out=h_res[0:64, col_base + t0:col_base + t0 + w],
                        in_=s2[:, :w])
                    for m0 in range(0, w, 128):
                        mw = min(128, w - m0)
                        pt = mmpool.tile([128, 512], dt.bfloat16, tag="mm")
                        nc.tensor.matmul(pt[:mw, :64], s2[:, m0:m0 + mw],
                                         ident16[:, :], start=True, stop=True,
                                         is_transpose=True)
                        hc = epool.tile([128, 64], dt.float32, tag="ehc")
                        nc.scalar.activation(out=hc[:mw, :], in_=pt[:mw, :64],
                                             func=AF.Copy)
                        nc.sync.dma_start(
                            out=out[col_base + t0 + m0:col_base + t0 + m0 + mw,
                                    0:64],
                            in_=hc[:mw, :])

            encode(xT, e1w, 17, 128, e2w, e2b, TI * 128, 0)
            encode(xnT, n1w, 9, 64, n2w, n2b, TN * 128, TI * 128)

            for t in range(NT):
                emit_xphase(0, t)

            # ---------- layers ----------
            qn = 0
            for l in range(L):
                nc.gpsimd.collective_compute(
                    "AllGather", OP.bypass,
                    replica_groups=[list(range(NC))],
                    ins=[xp_own.ap().opt()], outs=[xp_full.ap().opt()])

                use_g = flags["ln_g"][l]
                use_b = flags["ln_b"][l]
                offs = [0, 0]
                for w in range(NW):
                    tiles = list(range(w * WT, min(NT, (w + 1) * WT)))
                    aggT = [apool.tile([128, 512], dt.float32, tag=f"agg{j}",
                                       name=f"agg{j}")
                            for j in range(3)]

                    def agg_slice(i, d):
                        slot = i * 2 + d
                        return aggT[slot // 8][:, (slot % 8) * 64:
                                               (slot % 8) * 64 + 64]

                    # one PSUM accumulation group per bank (zero regions are
                    # 2KB): first matmul into a bank starts the group (which
                    # marks the whole bank pending-zero), the last stops it.
                    seq = []
                    for dd in range(2):
                        gsz = gszs[dd]
                        for s in range(NC):
                            for i, t in enumerate(tiles):
                                for k2 in range(int(gsz[t, s]) // 128):
                                    seq.append(((i * 2 + dd) // 8,
                                                (dd, s, i, k2)))
                    first_mm = {}
                    last_mm = {}
                    for j, kk in seq:
                        if j not in first_mm:
                            first_mm[j] = kk
                        last_mm[j] = kk
                    first_mm = set(first_mm.values())
                    last_mm = set(last_mm.values())

                    for dd in range(2):
                        gsz = gszs[dd]
                        for s in range(NC):
                            u0 = offs[dd]
                            usz = int(gsz[tiles[0]:tiles[-1] + 1, s].sum())
                            dl = dpool.tile([128, 64], dt.bfloat16, tag="dloc")
                            nc.sync.dma_start(
                                out=dl[:, :usz // 128],
                                in_=dloc_d[dd][:, u0 // 128:(u0 + usz) // 128])
                            git = ipool.tile([128, 512], dt.int16, tag="git")
                            nc.sync.dma_start(
                                out=git[:, :usz // 16],
                                in_=gidx_d[dd][:, u0 // 16:(u0 + usz) // 16])
                            # gather calls
                            gts = []
                            k = 0
                            while k < usz:
                                csz = min(CALL_MAX, usz - k)
                                gt = gpool.tile([128, CALL_MAX // 128, 128],
                                                dt.bfloat16, tag="gat")
                                nc.gpsimd.dma_gather(
                                    out_ap=gt[:, :csz // 128, :],
                                    in_ap=xp_full[s * TAB:(s + 1) * TAB, :],
                                    idxs_ap=git[:, k // 16:(k + csz) // 16],
                                    num_idxs=csz, num_idxs_reg=csz,
                                    elem_size=128, elem_step=128,
                                    queue_num=qn % 4)
                                qn += 1
                                gts.append((k, csz, gt))
                                k += csz
                            # indicators per 1024-slot block
                            inds = []
                            b = 0
                            while b < usz:
                                bsz = min(1024, usz - b)
                                ind = npool.tile([128, 8, 128], dt.bfloat16,
                                                 tag="ind")
                                nc.vector.tensor_tensor(
                                    out=ind[:, :bsz // 128, :],
                                    in0=iota_sb[:, :bsz].rearrange(
                                        "p (a j) -> p a j", j=128),
                                    in1=dl[:, b // 128:(b + bsz) // 128]
                                        .broadcast_to([128, bsz // 128, 128]),
                                    op=OP.is_equal)
                                inds.append(ind)
                                b += 1024
                            # matmuls
                            rel = 0
                            for i, t in enumerate(tiles):
                                n128 = int(gsz[t, s]) // 128
                                for k2 in range(n128):
                                    r = rel + k2 * 128
                                    ci = r // CALL_MAX
                                    k0, csz0, gt0 = gts[ci]
                                    kk = (dd, s, i, k2)
                                    nc.tensor.matmul(
                                        agg_slice(i, dd),
                                        inds[r // 1024][:, (r % 1024) // 128, :],
                                        gt0[:, (r - k0) // 128,
                                            dd * 64:dd * 64 + 64],
                                        start=(kk in first_mm),
                                        stop=(kk in last_mm))
                                rel += n128 * 128
                            offs[dd] += usz

                    # ----- combine wave -----
                    nwt = len(tiles)
                    sums = spool.tile([128, WT], dt.float32, tag="sums")
                    sqs = spool.tile([128, WT], dt.float32, tag="sqs")
                    hss = []
                    for i, t in enumerate(tiles):
                        p2 = mmpool.tile([128, 512], dt.float32, tag="mm")
                        nc.tensor.matmul(p2[:, :128],
                                         h_res[:, t * 128:(t + 1) * 128],
                                         wcr_sb[:, l * 128:(l + 1) * 128],
                                         start=True, stop=True)
                        stf = mpool.tile([128, 64], dt.float32, tag="stf")
                        stv = mpool.tile([128, 64], dt.float32, tag="stv")
                        nc.scalar.activation(out=stf[:], in_=p2[:, 0:64],
                                             func=AF.Relu,
                                             scale=ifo[:, t:t + 1])
                        nc.scalar.activation(out=stv[:], in_=p2[:, 64:128],
                                             func=AF.Relu,
                                             scale=iro[:, t:t + 1])
                        af = mpool.tile([128, 64], dt.float32, tag="af")
                        ar = mpool.tile([128, 64], dt.float32, tag="ar")
                        nc.scalar.activation(out=af[:], in_=agg_slice(i, 0),
                                             func=AF.Copy,
                                             scale=dfo[:, t:t + 1])
                        nc.scalar.activation(out=ar[:], in_=agg_slice(i, 1),
                                             func=AF.Copy,
                                             scale=dro[:, t:t + 1])
                        h1 = mpool.tile([128, 64], dt.float32, tag="h1")
                        h2 = mpool.tile([128, 64], dt.float32, tag="h2")
                        hs = hpool.tile([128, 64], dt.float32, tag="hs")
                        nc.vector.tensor_tensor(out=h1[:], in0=af[:],
                                                in1=stf[:], op=OP.add)
                        nc.vector.tensor_tensor(out=h2[:], in0=ar[:],
                                                in1=stv[:], op=OP.add)
                        nc.vector.tensor_tensor(out=hs[:], in0=h1[:],
                                                in1=h2[:], op=OP.add)
                        sc1 = mpool.tile([128, 64], dt.float32, tag="sc1")
                        sc2 = mpool.tile([128, 64], dt.float32, tag="sc2")
                        nc.scalar.activation(out=sc1[:], in_=hs[:],
                                             func=AF.Identity,
                                             accum_out=sums[:, i:i + 1])
                        nc.scalar.activation(out=sc2[:], in_=hs[:],
                                             func=AF.Square,
                                             accum_out=sqs[:, i:i + 1])
                        hss.append(hs)
                    m8 = spool.tile([128, WT], dt.float32, tag="m8")
                    ex2 = spool.tile([128, WT], dt.float32, tag="ex2")
                    nc.vector.tensor_scalar(out=m8[:, :nwt], in0=sums[:, :nwt],
                                            scalar1=1.0 / 64, scalar2=None,
                                            op0=OP.mult)
                    nc.vector.tensor_scalar(out=ex2[:, :nwt], in0=sqs[:, :nwt],
                                            scalar1=1.0 / 64, scalar2=None,
                                            op0=OP.mult)
                    msq = spool.tile([128, WT], dt.float32, tag="msq")
                    nc.vector.tensor_tensor(out=msq[:, :nwt], in0=m8[:, :nwt],
                                            in1=m8[:, :nwt], op=OP.mult)
                    var = spool.tile([128, WT], dt.float32, tag="var")
                    nc.vector.tensor_tensor(out=var[:, :nwt], in0=ex2[:, :nwt],
                                            in1=msq[:, :nwt], op=OP.subtract)
                    vpe = spool.tile([128, WT], dt.float32, tag="vpe")
                    nc.vector.tensor_scalar(out=vpe[:, :nwt], in0=var[:, :nwt],
                                            scalar1=EPS, scalar2=None,
                                            op0=OP.add)
                    sd = spool.tile([128, WT], dt.float32, tag="sd")
                    nc.scalar.activation(out=sd[:, :nwt], in_=vpe[:, :nwt],
                                         func=AF.Sqrt)
                    rstd = spool.tile([128, WT], dt.float32, tag="rstd")
                    nc.vector.reciprocal(out=rstd[:, :nwt], in_=sd[:, :nwt])
                    for i, t in enumerate(tiles):
                        hs = hss[i]
                        nm = mpool.tile([128, 64], dt.float32, tag="nm")
                        nc.vector.tensor_scalar(
                            out=nm[:], in0=hs[:], scalar1=m8[:, i:i + 1],
                            scalar2=rstd[:, i:i + 1],
                            op0=OP.subtract, op1=OP.mult)
                        cur = nm
                        if use_g:
                            gm = mpool.tile([128, 64], dt.float32, tag="gm")
                            nc.vector.tensor_tensor(
                                out=gm[:], in0=cur[:],
                                in1=lng_sb[:, l * 64:(l + 1) * 64], op=OP.mult)
                            cur = gm
                        if use_b:
                            bm = mpool.tile([128, 64], dt.float32, tag="bm")
                            nc.vector.tensor_tensor(
                                out=bm[:], in0=cur[:],
                                in1=lnb_sb[:, l * 64:(l + 1) * 64], op=OP.add)
                            cur = bm
                        hn = mpool.tile([128, 64], dt.float32, tag="hn")
                        ltmp = mpool.tile([128, 64], dt.float32, tag="ltmp")
                        leaky(hn[:], cur[:], ltmp[:])
                        nc.sync.dma_start(
                            out=out[t * 128:(t + 1) * 128,
                                    (l + 1) * 64:(l + 2) * 64],
                            in_=hn[:])
                        if l < L - 1:
                            pt = mmpool.tile([128, 512], dt.float32, tag="mm")
                            nc.tensor.matmul(pt[:64, :128], hn[:],
                                             ident32[:, :], start=True,
                                             stop=True, is_transpose=True)
                            nc.scalar.activation(
                                out=h_res[0:64, t * 128:(t + 1) * 128],
                                in_=pt[:64, :128], func=AF.Copy)
                            emit_xphase(l + 1, t)

    nc.compile()
    return nc


# ---------------------------------------------------------------------------
# entry point
# ---------------------------------------------------------------------------

def kernel(**inputs):
    from concourse.bass_utils import run_bass_kernel_spmd

    meta, per_core = _prep(inputs)
    key = (meta["n_inst"], meta["n_net"], meta["S_f"], meta["S_r"],
           meta["gsz_f"].tobytes(), meta["gsz_r"].tobytes(),
           tuple(meta["flags"]["ln_g"]), tuple(meta["flags"]["ln_b"]))
    if key not in _CACHE:
        _CACHE.clear()
        _CACHE[key] = _build(meta)
    nc = _CACHE[key]

    res = run_bass_kernel_spmd(nc, per_core, core_ids=list(range(NC)))

    n_inst, n_net = meta["n_inst"], meta["n_net"]
    si, sn = meta["si"], meta["sn"]
    outp = np.empty((n_inst + n_net, (L + 1) * D), np.float32)
    ji = np.arange(si)
    ri = (ji % TI) * 128 + ji // TI
    jn = np.arange(sn)
    rn = (TI + jn % TN) * 128 + jn // TN
    for c in range(NC):
        oc = res.results[c]["out"]
        outp[c * si:(c + 1) * si] = oc[ri]
        outp[n_inst + c * sn:n_inst + (c + 1) * sn] = oc[rn]
    return outp


# revision 31
# speedup vs baseline: 2.8902x; 1.0339x over previous
"""Trainium2 Bass kernel for nn_GNN_node_30279519437414 (GNN message passing).

Self-contained: takes FULL inputs, shards across 8 NeuronCores internally,
returns the FULL output.

Strategy:
  - Nodes sharded across 8 cores; within a core, nodes are spread over 310
    destination tiles of 128 partition slots (~101 nodes/tile) so that the
    per-(tile, source-core) edge groups rarely exceed 128 edges.
  - h is kept resident in SBUF (feat-major, bf16).  Per layer each core
    computes its own x' = dis * relu(h @ Wcat) slice (node-major bf16,
    fwd|rev packed in 128 features = 256B rows), writes it to DRAM packed at
    104 rows/tile (so chunk-local gather indices fit int16), and the x'
    tables are AllGathered.
  - Edge phase: per destination wave of 12 tiles, per source chunk, a single
    dma_gather pulls the source rows of all edges (dest-tile-grouped, padded
    to 128-multiples, group sizes common across cores = max).  Aggregation
    is done on the tensor engine: a one-hot indicator matrix (built on the
    vector engine with is_equal against an iota) is multiplied with the
    gathered messages, accumulating in PSUM per destination tile.  No
    scatter-add, no HBM round trip for the aggregate.
  - Combine (self-term + degree scaling + LayerNorm + leaky) reads the PSUM
    aggregates directly and is fused with the next layer's x' computation.
"""

import sys

sys.path.insert(0, "/opt/trn_rl_repo")

import numpy as np
import ml_dtypes

BF16 = ml_dtypes.bfloat16

NC = 8
D = 64
L = 3
EPS = 1e-5
TI = 248          # instance tiles per core
TN = 62           # net tiles per core
NT = TI + TN      # 310 dest tiles per core
TP = 104          # table rows per tile (packed, 310*104 = 32240 <= int16)
TAB = NT * TP     # 32240 table rows per core
DST = NT * 128    # 39680 dest rows per core
WT = 12           # tiles per wave
NW = (NT + WT - 1) // WT
CALL_MAX = 1024

_CACHE = {}


# ---------------------------------------------------------------------------
# host-side preprocessing
# ---------------------------------------------------------------------------

def _wrap_idx_dram(arr):
    """[S] int16 -> [128, S//16] (16-partition wrap replicated x8)."""
    w = arr.reshape(-1, 16).T.copy()
    return np.ascontiguousarray(np.tile(w, (8, 1)))


def _node_coords(ids, n_inst, si, sn):
    """ref node ids -> (core, tile, pos)."""
    ids = np.asarray(ids, dtype=np.int64)
    is_net = ids >= n_inst
    r = ids - n_inst
    c = np.where(is_net, r // sn, ids // si)
    j = np.where(is_net, r - (r // sn) * sn, ids - (ids // si) * si)
    tile = np.where(is_net, TI + j % TN, j % TI)
    pos = np.where(is_net, j // TN, j // TI)
    return c, tile, pos


def _edge_plan_dir(s_core, s_tab, d_core, d_tile, d_pos):
    """Group edges by (dest core, dest tile, src core); group sizes are the
    max over dest cores, padded to 128.  Returns (gsz [NT,8], S, gidx [8,S]
    int16 chunk-local table rows, dloc [8,S] f32 dest offsets, pads=255)."""
    gid = d_tile * NC + s_core                       # [E]
    cnts = np.zeros((NC, NT * NC), np.int64)
    for c in range(NC):
        cnts[c] = np.bincount(gid[d_core == c], minlength=NT * NC)
    mx = cnts.max(axis=0)
    gsz = ((np.maximum(mx, 1) + 127) // 128) * 128   # [NT*8]

    # group offsets in (wave, chunk, tile) emission order
    w_of_t = np.arange(NT) // WT
    ordk = ((w_of_t[:, None] * NC + np.arange(NC)[None, :]) * NT
            + np.arange(NT)[:, None]).ravel()        # [NT*8] by (t, s)
    order = np.argsort(ordk, kind="stable")
    offs = np.zeros(NT * NC, np.int64)
    offs[order] = np.concatenate([[0], np.cumsum(gsz[order])[:-1]])
    S = int(gsz.sum())

    # per-edge rank within (dest core, group)
    k2 = d_core.astype(np.int64) * (NT * NC) + gid
    o = np.argsort(k2, kind="stable")
    ks = k2[o]
    newg = np.empty(len(ks), dtype=bool)
    newg[0] = True
    np.not_equal(ks[1:], ks[:-1], out=newg[1:])
    starts = np.flatnonzero(newg)
    cnt2 = np.diff(np.r_[starts, len(ks)])
    rank = np.empty(len(ks), np.int64)
    rank[o] = np.arange(len(ks)) - np.repeat(starts, cnt2)
    slot = offs[gid] + rank

    gidx = np.zeros((NC, S), np.int16)
    dloc = np.full((NC, S), 255.0, np.float32)
    gidx[d_core, slot] = s_tab.astype(np.int16)
    dloc[d_core, slot] = d_pos
    return gsz.reshape(NT, NC), S, gidx, dloc


def _prep(inputs):
    n_inst = inputs["x"].shape[0]
    n_net = inputs["x_net"].shape[0]
    si, sn = n_inst // NC, n_net // NC
    N = n_inst + n_net
    assert si <= TI * 128 and sn <= TN * 128
    assert (si + TI - 1) // TI <= TP and (sn + TN - 1) // TN <= TP

    f = lambda k: np.asarray(inputs[k], dtype=np.float32)
    edge_index = inputs["edge_index"]
    row = np.asarray(edge_index[0], dtype=np.int64)
    col = np.asarray(edge_index[1], dtype=np.int64)

    deg_f = (np.bincount(row, minlength=N) + 1).astype(np.float32)
    deg_r = (np.bincount(col, minlength=N) + 1).astype(np.float32)
    dis_f = deg_f ** -0.5
    dis_r = deg_r ** -0.5
    inv_f = (1.0 / deg_f).astype(np.float32)
    inv_r = (1.0 / deg_r).astype(np.float32)

    # per-node dest coordinates for all ref ids
    allc, allt, allp = _node_coords(np.arange(N), n_inst, si, sn)
    drow = allt * 128 + allp                         # dest row within core

    def tabize(a):
        t = np.ones((NC, DST), np.float32)
        t[allc, drow] = a
        return t.reshape(NC, NT, 128).transpose(0, 2, 1).copy()  # [NC,128,NT]

    disf_t = tabize(dis_f)
    disr_t = tabize(dis_r)
    invf_t = tabize(inv_f)
    invr_t = tabize(inv_r)

    # edge plans
    rc, rt, rp = _node_coords(row, n_inst, si, sn)
    cc, ct, cp = _node_coords(col, n_inst, si, sn)
    rtab = (rt * TP + rp).astype(np.int64)           # chunk-local table row
    ctab = (ct * TP + cp).astype(np.int64)
    gsz_f, S_f, gidx_f, dloc_f = _edge_plan_dir(rc, rtab, cc, ct, cp)
    gsz_r, S_r, gidx_r, dloc_r = _edge_plan_dir(cc, ctab, rc, rt, rp)

    # weights
    enc1_Wb = np.vstack([f("enc1_W"), f("enc1_b")[None, :]])      # [17, 128]
    net1_Wb = np.vstack([f("net1_W"), f("net1_b")[None, :]])      # [9, 64]
    enc2_W, enc2_b = f("enc2_W"), f("enc2_b")
    net2_W, net2_b = f("net2_W"), f("net2_b")
    conv_W, conv_b, conv_root = f("conv_W"), f("conv_b"), f("conv_root")
    re_W, re_b, re_root = f("re_W"), f("re_b"), f("re_root")
    ln_g, ln_b = f("ln_g"), f("ln_b")

    wcat = np.zeros((L, 65, 128), np.float32)
    wcat_root = np.zeros((L, 65, 128), np.float32)
    for l in range(L):
        wcat[l, :64, :64] = conv_W[l]
        wcat[l, :64, 64:] = re_W[l]
        wcat[l, 64, :64] = conv_b[l]
        wcat[l, 64, 64:] = re_b[l]
        wcat_root[l] = wcat[l]
        wcat_root[l, 64, :64] += conv_root[l]
        wcat_root[l, 64, 64:] += re_root[l]

    flags = {
        "ln_g": [not np.allclose(ln_g[l], 1.0) for l in range(L)],
        "ln_b": [not np.allclose(ln_b[l], 0.0) for l in range(L)],
    }

    # encoder inputs in table-column order
    x = f("x")
    x_net = f("x_net")
    iota = np.tile(np.arange(128, dtype=np.float32), (128, 8))    # [128,1024]

    per_core = []
    for c in range(NC):
        xT = np.zeros((17, TI * 128), np.float32)
        jj = np.arange(si)
        xT[:16, (jj % TI) * 128 + jj // TI] = x[c * si:(c + 1) * si].T
        xT[16, :] = 1.0
        xnT = np.zeros((9, TN * 128), np.float32)
        jj = np.arange(sn)
        xnT[:8, (jj % TN) * 128 + jj // TN] = x_net[c * sn:(c + 1) * sn].T
        xnT[8, :] = 1.0
        d = {
            "xT": xT.astype(BF16),
            "xnT": xnT.astype(BF16),
            "disf": np.ascontiguousarray(disf_t[c]),
            "disr": np.ascontiguousarray(disr_t[c]),
            "invf": np.ascontiguousarray(invf_t[c]),
            "invr": np.ascontiguousarray(invr_t[c]),
            "gidx_f": _wrap_idx_dram(gidx_f[c]),
            "dloc_f": np.ascontiguousarray(
                dloc_f[c].reshape(-1, 128).T),
            "gidx_r": _wrap_idx_dram(gidx_r[c]),
            "dloc_r": np.ascontiguousarray(
                dloc_r[c].reshape(-1, 128).T),
            # replicated
            "enc1_Wb": enc1_Wb.astype(BF16),
            "enc2_W": np.ascontiguousarray(enc2_W).astype(BF16),
            "enc2_b": enc2_b.reshape(64, 1).copy(),
            "net1_Wb": net1_Wb.astype(BF16),
            "net2_W": np.ascontiguousarray(net2_W).astype(BF16),
            "net2_b": net2_b.reshape(64, 1).copy(),
            "wcat": wcat.astype(BF16),
            "wcat_root": wcat_root.astype(BF16),
            "iota": iota,
            "ln_g": np.ascontiguousarray(
                np.broadcast_to(ln_g[:, None, :], (L, 128, 64))),
            "ln_b": np.ascontiguousarray(
                np.broadcast_to(ln_b[:, None, :], (L, 128, 64))),
        }
        per_core.append(d)

    meta = {
        "n_inst": n_inst, "n_net": n_net, "si": si, "sn": sn,
        "gsz_f": gsz_f, "S_f": S_f, "gsz_r": gsz_r, "S_r": S_r,
        "flags": flags,
    }
    return meta, per_core


# ---------------------------------------------------------------------------
# device program
# ---------------------------------------------------------------------------

def _patch_lane_assignment():
    """Make Tile's DMASW lane choice queue-aware (queue q owns lanes 2q/2q+1)
    so SWDGE-queue round-robin doesn't trip the lane<->queue lock."""
    import concourse.tile_sem_assignment as tsa
    import concourse.mybir as mybir
    import concourse.bass_isa as bass_isa
    if getattr(tsa.TileClockTick, "_q_aware", False):
        return
    orig = tsa.TileClockTick._assign_tick

    def _assign_tick(self, inst):
        if (isinstance(inst, tsa.DMAInst)
                and not isinstance(inst, bass_isa.UserSyncedRemoteDMADescs)
                and inst.engine == mybir.EngineType.Pool
                and self.swdge_sem_count == tsa.NUM_SWDGE_GLOBAL_SEMS):
            qn = getattr(inst, "queue_num", 0) or 0
            if not hasattr(self, "_q_rr"):
                self._q_rr = {}
            r = self._q_rr.get(qn, 0)
            self._q_rr[qn] = r ^ 1
            self.next_sw_dma_idx = (qn * 2 + r) % self.swdge_sem_count
        return orig(self, inst)

    tsa.TileClockTick._assign_tick = _assign_tick
    tsa.TileClockTick._q_aware = True


def _build(meta):
    import concourse.bass as bass
    import concourse.bacc as bacc
    import concourse.mybir as mybir
    from concourse import tile
    from concourse import masks as _masks

    _patch_lane_assignment()

    dt = mybir.dt
    AF = mybir.ActivationFunctionType
    OP = mybir.AluOpType

    gszs = [meta["gsz_f"], meta["gsz_r"]]
    Ss = [meta["S_f"], meta["S_r"]]
    flags = meta["flags"]

    nc = bacc.Bacc("TRN2", target_bir_lowering=False, debug=False,
                   num_devices=NC, num_swdge_queues=4)

    ein = lambda n, s, d=dt.float32: nc.dram_tensor(n, s, d, kind="ExternalInput")
    xT = ein("xT", [17, TI * 128], dt.bfloat16)
    xnT = ein("xnT", [9, TN * 128], dt.bfloat16)
    disf = ein("disf", [128, NT]); disr = ein("disr", [128, NT])
    invf = ein("invf", [128, NT]); invr = ein("invr", [128, NT])
    gidx_d = [ein("gidx_f", [128, Ss[0] // 16], dt.int16),
              ein("gidx_r", [128, Ss[1] // 16], dt.int16)]
    dloc_d = [ein("dloc_f", [128, Ss[0] // 128]),
              ein("dloc_r", [128, Ss[1] // 128])]
    enc1_Wb = ein("enc1_Wb", [17, 128], dt.bfloat16)
    enc2_W = ein("enc2_W", [128, 64], dt.bfloat16)
    enc2_b = ein("enc2_b", [64, 1])
    net1_Wb = ein("net1_Wb", [9, 64], dt.bfloat16)
    net2_W = ein("net2_W", [64, 64], dt.bfloat16)
    net2_b = ein("net2_b", [64, 1])
    wcat_d = ein("wcat", [L, 65, 128], dt.bfloat16)
    wcatr_d = ein("wcat_root", [L, 65, 128], dt.bfloat16)
    iota_d = ein("iota", [128, 1024])
    ln_g_t = ein("ln_g", [L, 128, 64]); ln_b_t = ein("ln_b", [L, 128, 64])
    out = nc.dram_tensor("out", [DST, (L + 1) * D], dt.float32,
                         kind="ExternalOutput")

    xp_own = nc.dram_tensor("xp_own", [TAB, 128], dt.bfloat16)
    xp_full = nc.dram_tensor("xp_full", [NC * TAB, 128], dt.bfloat16,
                             addr_space="Shared")

    with tile.TileContext(nc) as tc:
        with (
            tc.tile_pool(name="const", bufs=1) as cpool,
            tc.tile_pool(name="enc", bufs=3) as epool,
            tc.tile_pool(name="idx", bufs=4) as ipool,
            tc.tile_pool(name="dlc", bufs=4) as dpool,
            tc.tile_pool(name="gat", bufs=6) as gpool,
            tc.tile_pool(name="ind", bufs=6) as npool,
            tc.tile_pool(name="cmb", bufs=4) as mpool,
            tc.tile_pool(name="hst", bufs=WT + 3) as hpool,
            tc.tile_pool(name="sml", bufs=3) as spool,
            tc.tile_pool(name="xps", bufs=3) as xpool,
            tc.tile_pool(name="agg", bufs=2, space="PSUM") as apool,
            tc.tile_pool(name="pmm", bufs=2, space="PSUM") as mmpool,
        ):
            # ---------- constants ----------
            h_res = cpool.tile([65, NT * 128], dt.bfloat16)
            for o in range(0, NT * 128, 4096):
                ww = min(4096, NT * 128 - o)
                nc.vector.memset(h_res[64:65, o:o + ww], 1.0)
            dfo = cpool.tile([128, NT], dt.float32)
            dro = cpool.tile([128, NT], dt.float32)
            ifo = cpool.tile([128, NT], dt.float32)
            iro = cpool.tile([128, NT], dt.float32)
            nc.sync.dma_start(out=dfo[:], in_=disf[:, :])
            nc.sync.dma_start(out=dro[:], in_=disr[:, :])
            nc.sync.dma_start(out=ifo[:], in_=invf[:, :])
            nc.sync.dma_start(out=iro[:], in_=invr[:, :])
            e1w = cpool.tile([17, 128], dt.bfloat16)
            e2w = cpool.tile([128, 64], dt.bfloat16)
            e2b = cpool.tile([64, 1], dt.float32)
            n1w = cpool.tile([9, 64], dt.bfloat16)
            n2w = cpool.tile([64, 64], dt.bfloat16)
            n2b = cpool.tile([64, 1], dt.float32)
            nc.sync.dma_start(out=e1w[:], in_=enc1_Wb[:, :])
            nc.sync.dma_start(out=e2w[:], in_=enc2_W[:, :])
            nc.sync.dma_start(out=e2b[:], in_=enc2_b[:, :])
            nc.sync.dma_start(out=n1w[:], in_=net1_Wb[:, :])
            nc.sync.dma_start(out=n2w[:], in_=net2_W[:, :])
            nc.sync.dma_start(out=n2b[:], in_=net2_b[:, :])
            wc_sb = cpool.tile([65, L * 128], dt.bfloat16)
            wcr_sb = cpool.tile([65, L * 128], dt.bfloat16)
            nc.sync.dma_start(
                out=wc_sb[:].rearrange("p (l d) -> p l d", l=L),
                in_=wcat_d.ap().rearrange("l p d -> p l d"))
            nc.sync.dma_start(
                out=wcr_sb[:].rearrange("p (l d) -> p l d", l=L),
                in_=wcatr_d.ap().rearrange("l p d -> p l d"))
            iota_sb = cpool.tile([128, 1024], dt.float32)
            nc.sync.dma_start(out=iota_sb[:], in_=iota_d[:, :])
            lng_sb = cpool.tile([128, L * 64], dt.float32)
            lnb_sb = cpool.tile([128, L * 64], dt.float32)
            nc.sync.dma_start(
                out=lng_sb[:].rearrange("p (l d) -> p l d", l=L),
                in_=ln_g_t.ap().rearrange("l p d -> p l d"))
            nc.sync.dma_start(
                out=lnb_sb[:].rearrange("p (l d) -> p l d", l=L),
                in_=ln_b_t.ap().rearrange("l p d -> p l d"))
            ident32 = cpool.tile([128, 128], dt.float32)
            _masks.make_identity(nc, ident32[:])
            ident16 = cpool.tile([64, 64], dt.bfloat16)
            _masks.make_identity(nc, ident16[:])
            # pre-zero the gather pool bufs: pad slots use negative (skipped)
            # gather indices, so whatever is in the buffer must be finite for
            # the 0-weighted matmul contribution to stay 0.
            for _z in range(6):
                gz = gpool.tile([128, CALL_MAX // 128, 128], dt.bfloat16,
                                tag="gat", name=f"gz{_z}")
                nc.vector.memset(gz[:], 0.0)

            def leaky(dst_ap, src_ap, tmp_tile):
                nc.vector.tensor_scalar(out=tmp_tile, in0=src_ap, scalar1=0.1,
                                        scalar2=None, op0=OP.mult)
                nc.vector.tensor_tensor(out=dst_ap, in0=src_ap, in1=tmp_tile,
                                        op=OP.max)

            # ---------- x' phase for one tile ----------
            def emit_xphase(l, t):
                px = mmpool.tile([128, 512], dt.float32, tag="mm")
                nc.tensor.matmul(px[:, :128],
                                 h_res[:, t * 128:(t + 1) * 128],
                                 wc_sb[:, l * 128:(l + 1) * 128],
                                 start=True, stop=True)
                xps = xpool.tile([128, 128], dt.bfloat16, tag="xps")
                nc.scalar.activation(out=xps[:, 0:64], in_=px[:, 0:64],
                                     func=AF.Relu, scale=dfo[:, t:t + 1])
                nc.scalar.activation(out=xps[:, 64:128], in_=px[:, 64:128],
                                     func=AF.Relu, scale=dro[:, t:t + 1])
                nc.sync.dma_start(out=xp_own[t * TP:(t + 1) * TP, :],
                                  in_=xps[0:TP, :])

            # ---------- encoder ----------
            def encode(inpT, w1, nf1, nmid, w2, b2, ncols, col_base):
                for t0 in range(0, ncols, 512):
                    w = min(512, ncols - t0)
                    rhs = epool.tile([nf1, 512], dt.bfloat16, tag="erhs")
                    nc.sync.dma_start(out=rhs[:, :w], in_=inpT[:, t0:t0 + w])
                    p1 = mmpool.tile([128, 512], dt.float32, tag="mm")
                    nc.tensor.matmul(p1[:nmid, :w], w1[:], rhs[:nf1, :w],
                                     start=True, stop=True)
                    s1 = epool.tile([128, 512], dt.bfloat16, tag="es1")
                    tmp1 = epool.tile([128, 512], dt.float32, tag="etmp1")
                    leaky(s1[:nmid, :w], p1[:nmid, :w], tmp1[:nmid, :w])
                    p2 = mmpool.tile([128, 512], dt.float32, tag="mm")
                    nc.tensor.matmul(p2[:64, :w], w2[:], s1[:nmid, :w],
                                     start=True, stop=True)
                    s2 = epool.tile([64, 512], dt.bfloat16, tag="es2")
                    badd = epool.tile([64, 512], dt.float32, tag="ebadd")
                    nc.vector.tensor_scalar(out=badd[:, :w], in0=p2[:64, :w],
                                            scalar1=b2[:, 0:1], scalar2=None,
                                            op0=OP.add)
                    tmp2 = epool.tile([64, 512], dt.float32, tag="etmp2")
                    leaky(s2[:, :w], badd[:, :w], tmp2[:, :w])
                    nc.vector.tensor_copy(
                        out=h_res[0:64, col_base + t0:col_base + t0 + w],
                        in_=s2[:, :w])
                    for m0 in range(0, w, 128):
                        mw = min(128, w - m0)
                        pt = mmpool.tile([128, 512], dt.bfloat16, tag="mm")
                        nc.tensor.matmul(pt[:mw, :64], s2[:, m0:m0 + mw],
                                         ident16[:, :], start=True, stop=True,
                                         is_transpose=True)
                        hc = epool.tile([128, 64], dt.float32, tag="ehc")
                        nc.scalar.activation(out=hc[:mw, :], in_=pt[:mw, :64],
                                             func=AF.Copy)
                        nc.sync.dma_start(
                            out=out[col_base + t0 + m0:col_base + t0 + m0 + mw,
                                    0:64],
                            in_=hc[:mw, :])

            encode(xT, e1w, 17, 128, e2w, e2b, TI * 128, 0)
            encode(xnT, n1w, 9, 64, n2w, n2b, TN * 128, TI * 128)

            for t in range(NT):
                emit_xphase(0, t)

            # ---------- layers ----------
            qn = 0
            for l in range(L):
                nc.gpsimd.collective_compute(
                    "AllGather", OP.bypass,
                    replica_groups=[list(range(NC))],
                    ins=[xp_own.ap().opt()], outs=[xp_full.ap().opt()])

                use_g = flags["ln_g"][l]
                use_b = flags["ln_b"][l]
                offs = [0, 0]
                for w in range(NW):
                    tiles = list(range(w * WT, min(NT, (w + 1) * WT)))
                    aggT = [apool.tile([128, 512], dt.float32, tag=f"agg{j}",
                                       name=f"agg{j}")
                            for j in range(3)]

                    def agg_slice(i, d):
                        slot = i * 2 + d
                        return aggT[slot // 8][:, (slot % 8) * 64:
                                               (slot % 8) * 64 + 64]

                    # one PSUM accumulation group per bank (zero regions are
                    # 2KB): first matmul into a bank starts the group (which
                    # marks the whole bank pending-zero), the last stops it.
                    seq = []
                    for dd in range(2):
                        gsz = gszs[dd]
                        for s in range(NC):
                            for i, t in enumerate(tiles):
                                for k2 in range(int(gsz[t, s]) // 128):
                                    seq.append(((i * 2 + dd) // 8,
                                                (dd, s, i, k2)))
                    first_mm = {}
                    last_mm = {}
                    for j, kk in seq:
                        if j not in first_mm:
                            first_mm[j] = kk
                        last_mm[j] = kk
                    first_mm = set(first_mm.values())
                    last_mm = set(last_mm.values())

                    for dd in range(2):
                        gsz = gszs[dd]
                        for s in range(NC):
                            u0 = offs[dd]
                            usz = int(gsz[tiles[0]:tiles[-1] + 1, s].sum())
                            dl = dpool.tile([128, 64], dt.float32, tag="dloc")
                            nc.sync.dma_start(
                                out=dl[:, :usz // 128],
                                in_=dloc_d[dd][:, u0 // 128:(u0 + usz) // 128])
                            git = ipool.tile([128, 512], dt.int16, tag="git")
                            nc.sync.dma_start(
                                out=git[:, :usz // 16],
                                in_=gidx_d[dd][:, u0 // 16:(u0 + usz) // 16])
                            # gather calls
                            gts = []
                            k = 0
                            while k < usz:
                                csz = min(CALL_MAX, usz - k)
                                gt = gpool.tile([128, CALL_MAX // 128, 128],
                                                dt.bfloat16, tag="gat")
                                nc.gpsimd.dma_gather(
                                    out_ap=gt[:, :csz // 128, :],
                                    in_ap=xp_full[s * TAB:(s + 1) * TAB, :],
                                    idxs_ap=git[:, k // 16:(k + csz) // 16],
                                    num_idxs=csz, num_idxs_reg=csz,
                                    elem_size=128, elem_step=128,
                                    queue_num=qn % 4)
                                qn += 1
                                gts.append((k, csz, gt))
                                k += csz
                            # indicators per 1024-slot block
                            inds = []
                            b = 0
                            while b < usz:
                                bsz = min(1024, usz - b)
                                ind = npool.tile([128, 8, 128], dt.bfloat16,
                                                 tag="ind")
                                nc.vector.tensor_tensor(
                                    out=ind[:, :bsz // 128, :],
                                    in0=iota_sb[:, :bsz].rearrange(
                                        "p (a j) -> p a j", j=128),
                                    in1=dl[:, b // 128:(b + bsz) // 128]
                                        .broadcast_to([128, bsz // 128, 128]),
                                    op=OP.is_equal)
                                inds.append(ind)
                                b += 1024
                            # matmuls
                            rel = 0
                            for i, t in enumerate(tiles):
                                n128 = int(gsz[t, s]) // 128
                                for k2 in range(n128):
                                    r = rel + k2 * 128
                                    ci = r // CALL_MAX
                                    k0, csz0, gt0 = gts[ci]
                                    kk = (dd, s, i, k2)
                                    nc.tensor.matmul(
                                        agg_slice(i, dd),
                                        inds[r // 1024][:, (r % 1024) // 128, :],
                                        gt0[:, (r - k0) // 128,
                                            dd * 64:dd * 64 + 64],
                                        start=(kk in first_mm),
                                        stop=(kk in last_mm))
                                rel += n128 * 128
                            offs[dd] += usz

                    # ----- combine wave -----
                    nwt = len(tiles)
                    sums = spool.tile([128, WT], dt.float32, tag="sums")
                    sqs = spool.tile([128, WT], dt.float32, tag="sqs")
                    hss = []
                    for i, t in enumerate(tiles):
                        p2 = mmpool.tile([128, 512], dt.float32, tag="mm")
                        nc.tensor.matmul(p2[:, :128],
                                         h_res[:, t * 128:(t + 1) * 128],
                                         wcr_sb[:, l * 128:(l + 1) * 128],
                                         start=True, stop=True)
                        stf = mpool.tile([128, 64], dt.float32, tag="stf")
                        stv = mpool.tile([128, 64], dt.float32, tag="stv")
                        nc.scalar.activation(out=stf[:], in_=p2[:, 0:64],
                                             func=AF.Relu,
                                             scale=ifo[:, t:t + 1])
                        nc.scalar.activation(out=stv[:], in_=p2[:, 64:128],
                                             func=AF.Relu,
                                             scale=iro[:, t:t + 1])
                        af = mpool.tile([128, 64], dt.float32, tag="af")
                        ar = mpool.tile([128, 64], dt.float32, tag="ar")
                        nc.scalar.activation(out=af[:], in_=agg_slice(i, 0),
                                             func=AF.Copy,
                                             scale=dfo[:, t:t + 1])
                        nc.scalar.activation(out=ar[:], in_=agg_slice(i, 1),
                                             func=AF.Copy,
                                             scale=dro[:, t:t + 1])
                        h1 = mpool.tile([128, 64], dt.float32, tag="h1")
                        h2 = mpool.tile([128, 64], dt.float32, tag="h2")
                        hs = hpool.tile([128, 64], dt.float32, tag="hs")
                        nc.vector.tensor_tensor(out=h1[:], in0=af[:],
                                                in1=stf[:], op=OP.add)
                        nc.vector.tensor_tensor(out=h2[:], in0=ar[:],
                                                in1=stv[:], op=OP.add)
                        nc.vector.tensor_tensor(out=hs[:], in0=h1[:],
                                                in1=h2[:], op=OP.add)
                        sc1 = mpool.tile([128, 64], dt.float32, tag="sc1")
                        sc2 = mpool.tile([128, 64], dt.float32, tag="sc2")
                        nc.scalar.activation(out=sc1[:], in_=hs[:],
                                             func=AF.Identity,
                                             accum_out=sums[:, i:i + 1])
                        nc.scalar.activation(out=sc2[:], in_=hs[:],
                                             func=AF.Square,
                                             accum_out=sqs[:, i:i + 1])
                        hss.append(hs)
                    m8 = spool.tile([128, WT], dt.float32, tag="m8")
                    ex2 = spool.tile([128, WT], dt.float32, tag="ex2")
                    nc.vector.tensor_scalar(out=m8[:, :nwt], in0=sums[:, :nwt],
                                            scalar1=1.0 / 64, scalar2=None,
                                            op0=OP.mult)
                    nc.vector.tensor_scalar(out=ex2[:, :nwt], in0=sqs[:, :nwt],
                                            scalar1=1.0 / 64, scalar2=None,
                                            op0=OP.mult)
                    msq = spool.tile([128, WT], dt.float32, tag="msq")
                    nc.vector.tensor_tensor(out=msq[:, :nwt], in0=m8[:, :nwt],
                                            in1=m8[:, :nwt], op=OP.mult)
                    var = spool.tile([128, WT], dt.float32, tag="var")
                    nc.vector.tensor_tensor(out=var[:, :nwt], in0=ex2[:, :nwt],
                                            in1=msq[:, :nwt], op=OP.subtract)
                    vpe = spool.tile([128, WT], dt.float32, tag="vpe")
                    nc.vector.tensor_scalar(out=vpe[:, :nwt], in0=var[:, :nwt],
                                            scalar1=EPS, scalar2=None,
                                            op0=OP.add)
                    sd = spool.tile([128, WT], dt.float32, tag="sd")
                    nc.scalar.activation(out=sd[:, :nwt], in_=vpe[:, :nwt],
                                         func=AF.Sqrt)
                    rstd = spool.tile([128, WT], dt.float32, tag="rstd")
                    nc.vector.reciprocal(out=rstd[:, :nwt], in_=sd[:, :nwt])
                    nmr = spool.tile([128, WT], dt.float32, tag="nmr")
                    nc.vector.tensor_scalar(out=nmr[:, :nwt], in0=m8[:, :nwt],
                                            scalar1=-1.0, scalar2=None,
                                            op0=OP.mult)
                    nc.vector.tensor_tensor(out=nmr[:, :nwt], in0=nmr[:, :nwt],
                                            in1=rstd[:, :nwt], op=OP.mult)
                    for i, t in enumerate(tiles):
                        hs = hss[i]
                        nm = mpool.tile([128, 64], dt.float32, tag="nm")
                        nc.scalar.activation(out=nm[:], in_=hs[:],
                                             func=AF.Identity,
                                             scale=rstd[:, i:i + 1],
                                             bias=nmr[:, i:i + 1])
                        cur = nm
                        if use_g:
                            gm = mpool.tile([128, 64], dt.float32, tag="gm")
                            nc.vector.tensor_tensor(
                                out=gm[:], in0=cur[:],
                                in1=lng_sb[:, l * 64:(l + 1) * 64], op=OP.mult)
                            cur = gm
                        if use_b:
                            bm = mpool.tile([128, 64], dt.float32, tag="bm")
                            nc.vector.tensor_tensor(
                                out=bm[:], in0=cur[:],
                                in1=lnb_sb[:, l * 64:(l + 1) * 64], op=OP.add)
                            cur = bm
                        hn = mpool.tile([128, 64], dt.float32, tag="hn")
                        ltmp = mpool.tile([128, 64], dt.float32, tag="ltmp")
                        leaky(hn[:], cur[:], ltmp[:])
                        nc.sync.dma_start(
                            out=out[t * 128:(t + 1) * 128,
                                    (l + 1) * 64:(l + 2) * 64],
                            in_=hn[:])
                        if l < L - 1:
                            pt = mmpool.tile([128, 512], dt.float32, tag="mm")
                            nc.tensor.matmul(pt[:64, :128], hn[:],
                                             ident32[:, :], start=True,
                                             stop=True, is_transpose=True)
                            nc.scalar.activation(
                                out=h_res[0:64, t * 128:(t + 1) * 128],
                                in_=pt[:64, :128], func=AF.Copy)
                            emit_xphase(l + 1, t)

    nc.compile()
    return nc


# ---------------------------------------------------------------------------
# entry point
# ---------------------------------------------------------------------------

def kernel(**inputs):
    from concourse.bass_utils import run_bass_kernel_spmd

    meta, per_core = _prep(inputs)
    key = (meta["n_inst"], meta["n_net"], meta["S_f"], meta["S_r"],
           meta["gsz_f"].tobytes(), meta["gsz_r"].tobytes(),
           tuple(meta["flags"]["ln_g"]), tuple(meta["flags"]["ln_b"]))
    if key not in _CACHE:
        _CACHE.clear()
        _CACHE[key] = _build(meta)
    nc = _CACHE[key]

    res = run_bass_kernel_spmd(nc, per_core, core_ids=list(range(NC)))

    n_inst, n_net = meta["n_inst"], meta["n_net"]
    si, sn = meta["si"], meta["sn"]
    outp = np.empty((n_inst + n_net, (L + 1) * D), np.float32)
    ji = np.arange(si)
    ri = (ji % TI) * 128 + ji // TI
    jn = np.arange(sn)
    rn = (TI + jn % TN) * 128 + jn // TN
    for c in range(NC):
        oc = res.results[c]["out"]
        outp[c * si:(c + 1) * si] = oc[ri]
        outp[n_inst + c * sn:n_inst + (c + 1) * sn] = oc[rn]
    return outp


# revision 35
# speedup vs baseline: 2.9937x; 1.0358x over previous
"""Trainium2 Bass kernel for nn_GNN_node_30279519437414 (GNN message passing).

Self-contained: takes FULL inputs, shards across 8 NeuronCores internally,
returns the FULL output.

Strategy:
  - Nodes sharded across 8 cores; within a core, nodes are spread over 310
    destination tiles of 128 partition slots (~101 nodes/tile) so that the
    per-(tile, source-core) edge groups rarely exceed 128 edges.
  - h is kept resident in SBUF (feat-major, bf16).  Per layer each core
    computes its own x' = dis * relu(h @ Wcat) slice (node-major bf16,
    fwd|rev packed in 128 features = 256B rows), writes it to DRAM packed at
    104 rows/tile (so chunk-local gather indices fit int16), and the x'
    tables are AllGathered.
  - Edge phase: per destination wave of 12 tiles, per source chunk, a single
    dma_gather pulls the source rows of all edges (dest-tile-grouped, padded
    to 128-multiples, group sizes common across cores = max).  Aggregation
    is done on the tensor engine: a one-hot indicator matrix (built on the
    vector engine with is_equal against an iota) is multiplied with the
    gathered messages, accumulating in PSUM per destination tile.  No
    scatter-add, no HBM round trip for the aggregate.
  - Combine (self-term + degree scaling + LayerNorm + leaky) reads the PSUM
    aggregates directly and is fused with the next layer's x' computation.
"""

import sys

sys.path.insert(0, "/opt/trn_rl_repo")

import numpy as np
import ml_dtypes

BF16 = ml_dtypes.bfloat16

NC = 8
D = 64
L = 3
EPS = 1e-5
TI = 248          # instance tiles per core
TN = 62           # net tiles per core
NT = TI + TN      # 310 dest tiles per core
TP = 104          # table rows per tile (packed, 310*104 = 32240 <= int16)
TAB = NT * TP     # 32240 table rows per core
DST = NT * 128    # 39680 dest rows per core
WT = 12           # tiles per wave
NW = (NT + WT - 1) // WT
CALL_MAX = 1024

_CACHE = {}


# ---------------------------------------------------------------------------
# host-side preprocessing
# ---------------------------------------------------------------------------

def _wrap_idx_dram(arr):
    """[S] int16 -> [128, S//16] (16-partition wrap replicated x8)."""
    w = arr.reshape(-1, 16).T.copy()
    return np.ascontiguousarray(np.tile(w, (8, 1)))


def _node_coords(ids, n_inst, si, sn):
    """ref node ids -> (core, tile, pos)."""
    ids = np.asarray(ids, dtype=np.int64)
    is_net = ids >= n_inst
    r = ids - n_inst
    c = np.where(is_net, r // sn, ids // si)
    j = np.where(is_net, r - (r // sn) * sn, ids - (ids // si) * si)
    tile = np.where(is_net, TI + j % TN, j % TI)
    pos = np.where(is_net, j // TN, j // TI)
    return c, tile, pos


def _edge_plan_dir(s_core, s_tab, d_core, d_tile, d_pos):
    """Group edges by (dest core, dest tile, src core); group sizes are the
    max over dest cores, padded to 128.  Returns (gsz [NT,8], S, gidx [8,S]
    int16 chunk-local table rows, dloc [8,S] f32 dest offsets, pads=255)."""
    gid = d_tile * NC + s_core                       # [E]
    cnts = np.zeros((NC, NT * NC), np.int64)
    for c in range(NC):
        cnts[c] = np.bincount(gid[d_core == c], minlength=NT * NC)
    mx = cnts.max(axis=0)
    gsz = ((np.maximum(mx, 1) + 127) // 128) * 128   # [NT*8]

    # group offsets in (wave, chunk, tile) emission order
    w_of_t = np.arange(NT) // WT
    ordk = ((w_of_t[:, None] * NC + np.arange(NC)[None, :]) * NT
            + np.arange(NT)[:, None]).ravel()        # [NT*8] by (t, s)
    order = np.argsort(ordk, kind="stable")
    offs = np.zeros(NT * NC, np.int64)
    offs[order] = np.concatenate([[0], np.cumsum(gsz[order])[:-1]])
    S = int(gsz.sum())

    # per-edge rank within (dest core, group)
    k2 = d_core.astype(np.int64) * (NT * NC) + gid
    o = np.argsort(k2, kind="stable")
    ks = k2[o]
    newg = np.empty(len(ks), dtype=bool)
    newg[0] = True
    np.not_equal(ks[1:], ks[:-1], out=newg[1:])
    starts = np.flatnonzero(newg)
    cnt2 = np.diff(np.r_[starts, len(ks)])
    rank = np.empty(len(ks), np.int64)
    rank[o] = np.arange(len(ks)) - np.repeat(starts, cnt2)
    slot = offs[gid] + rank

    gidx = np.zeros((NC, S), np.int16)
    gidx[d_core, slot] = s_tab.astype(np.int16)
    # one-hot indicator bytes (fp8 e4m3: 1.0 = 0x38), wrapped [128, S]
    inds = []
    for c in range(NC):
        m = d_core == c
        ind = np.zeros((S, 128), np.uint8)
        ind[slot[m], d_pos[m]] = 0x38
        inds.append(np.ascontiguousarray(
            ind.reshape(S // 128, 128, 128).transpose(1, 0, 2)
               .reshape(128, S)))
    return gsz.reshape(NT, NC), S, gidx, inds


def _prep(inputs):
    n_inst = inputs["x"].shape[0]
    n_net = inputs["x_net"].shape[0]
    si, sn = n_inst // NC, n_net // NC
    N = n_inst + n_net
    assert si <= TI * 128 and sn <= TN * 128
    assert (si + TI - 1) // TI <= TP and (sn + TN - 1) // TN <= TP

    f = lambda k: np.asarray(inputs[k], dtype=np.float32)
    edge_index = inputs["edge_index"]
    row = np.asarray(edge_index[0], dtype=np.int64)
    col = np.asarray(edge_index[1], dtype=np.int64)

    deg_f = (np.bincount(row, minlength=N) + 1).astype(np.float32)
    deg_r = (np.bincount(col, minlength=N) + 1).astype(np.float32)
    dis_f = deg_f ** -0.5
    dis_r = deg_r ** -0.5
    inv_f = (1.0 / deg_f).astype(np.float32)
    inv_r = (1.0 / deg_r).astype(np.float32)

    # per-node dest coordinates for all ref ids
    allc, allt, allp = _node_coords(np.arange(N), n_inst, si, sn)
    drow = allt * 128 + allp                         # dest row within core

    def tabize(a):
        t = np.ones((NC, DST), np.float32)
        t[allc, drow] = a
        return t.reshape(NC, NT, 128).transpose(0, 2, 1).copy()  # [NC,128,NT]

    disf_t = tabize(dis_f)
    disr_t = tabize(dis_r)
    invf_t = tabize(inv_f)
    invr_t = tabize(inv_r)

    # edge plans
    rc, rt, rp = _node_coords(row, n_inst, si, sn)
    cc, ct, cp = _node_coords(col, n_inst, si, sn)
    rtab = (rt * TP + rp).astype(np.int64)           # chunk-local table row
    ctab = (ct * TP + cp).astype(np.int64)
    gsz_f, S_f, gidx_f, ind_f = _edge_plan_dir(rc, rtab, cc, ct, cp)
    gsz_r, S_r, gidx_r, ind_r = _edge_plan_dir(cc, ctab, rc, rt, rp)

    # weights
    enc1_Wb = np.vstack([f("enc1_W"), f("enc1_b")[None, :]])      # [17, 128]
    net1_Wb = np.vstack([f("net1_W"), f("net1_b")[None, :]])      # [9, 64]
    enc2_W, enc2_b = f("enc2_W"), f("enc2_b")
    net2_W, net2_b = f("net2_W"), f("net2_b")
    conv_W, conv_b, conv_root = f("conv_W"), f("conv_b"), f("conv_root")
    re_W, re_b, re_root = f("re_W"), f("re_b"), f("re_root")
    ln_g, ln_b = f("ln_g"), f("ln_b")

    wcat = np.zeros((L, 65, 128), np.float32)
    wcat_root = np.zeros((L, 65, 128), np.float32)
    for l in range(L):
        wcat[l, :64, :64] = conv_W[l]
        wcat[l, :64, 64:] = re_W[l]
        wcat[l, 64, :64] = conv_b[l]
        wcat[l, 64, 64:] = re_b[l]
        wcat_root[l] = wcat[l]
        wcat_root[l, 64, :64] += conv_root[l]
        wcat_root[l, 64, 64:] += re_root[l]

    flags = {
        "ln_g": [not np.allclose(ln_g[l], 1.0) for l in range(L)],
        "ln_b": [not np.allclose(ln_b[l], 0.0) for l in range(L)],
    }

    # encoder inputs in table-column order
    x = f("x")
    x_net = f("x_net")

    per_core = []
    for c in range(NC):
        xT = np.zeros((17, TI * 128), np.float32)
        jj = np.arange(si)
        xT[:16, (jj % TI) * 128 + jj // TI] = x[c * si:(c + 1) * si].T
        xT[16, :] = 1.0
        xnT = np.zeros((9, TN * 128), np.float32)
        jj = np.arange(sn)
        xnT[:8, (jj % TN) * 128 + jj // TN] = x_net[c * sn:(c + 1) * sn].T
        xnT[8, :] = 1.0
        d = {
            "xT": xT.astype(BF16),
            "xnT": xnT.astype(BF16),
            "disf": np.ascontiguousarray(disf_t[c]),
            "disr": np.ascontiguousarray(disr_t[c]),
            "invf": np.ascontiguousarray(invf_t[c]),
            "invr": np.ascontiguousarray(invr_t[c]),
            "gidx_f": _wrap_idx_dram(gidx_f[c]),
            "ind_f": ind_f[c],
            "gidx_r": _wrap_idx_dram(gidx_r[c]),
            "ind_r": ind_r[c],
            # replicated
            "enc1_Wb": enc1_Wb.astype(BF16),
            "enc2_W": np.ascontiguousarray(enc2_W).astype(BF16),
            "enc2_b": enc2_b.reshape(64, 1).copy(),
            "net1_Wb": net1_Wb.astype(BF16),
            "net2_W": np.ascontiguousarray(net2_W).astype(BF16),
            "net2_b": net2_b.reshape(64, 1).copy(),
            "wcat": wcat.astype(BF16),
            "wcat_root": wcat_root.astype(BF16),
            "ln_g": np.ascontiguousarray(
                np.broadcast_to(ln_g[:, None, :], (L, 128, 64))),
            "ln_b": np.ascontiguousarray(
                np.broadcast_to(ln_b[:, None, :], (L, 128, 64))),
        }
        per_core.append(d)

    meta = {
        "n_inst": n_inst, "n_net": n_net, "si": si, "sn": sn,
        "gsz_f": gsz_f, "S_f": S_f, "gsz_r": gsz_r, "S_r": S_r,
        "flags": flags,
    }
    return meta, per_core


# ---------------------------------------------------------------------------
# device program
# ---------------------------------------------------------------------------

def _patch_lane_assignment():
    """Make Tile's DMASW lane choice queue-aware (queue q owns lanes 2q/2q+1)
    so SWDGE-queue round-robin doesn't trip the lane<->queue lock."""
    import concourse.tile_sem_assignment as tsa
    import concourse.mybir as mybir
    import concourse.bass_isa as bass_isa
    if getattr(tsa.TileClockTick, "_q_aware", False):
        return
    orig = tsa.TileClockTick._assign_tick

    def _assign_tick(self, inst):
        if (isinstance(inst, tsa.DMAInst)
                and not isinstance(inst, bass_isa.UserSyncedRemoteDMADescs)
                and inst.engine == mybir.EngineType.Pool
                and self.swdge_sem_count == tsa.NUM_SWDGE_GLOBAL_SEMS):
            qn = getattr(inst, "queue_num", 0) or 0
            if not hasattr(self, "_q_rr"):
                self._q_rr = {}
            r = self._q_rr.get(qn, 0)
            self._q_rr[qn] = r ^ 1
            self.next_sw_dma_idx = (qn * 2 + r) % self.swdge_sem_count
        return orig(self, inst)

    tsa.TileClockTick._assign_tick = _assign_tick
    tsa.TileClockTick._q_aware = True


def _build(meta):
    import concourse.bass as bass
    import concourse.bacc as bacc
    import concourse.mybir as mybir
    from concourse import tile
    from concourse import masks as _masks

    _patch_lane_assignment()

    dt = mybir.dt
    AF = mybir.ActivationFunctionType
    OP = mybir.AluOpType

    gszs = [meta["gsz_f"], meta["gsz_r"]]
    Ss = [meta["S_f"], meta["S_r"]]
    flags = meta["flags"]

    nc = bacc.Bacc("TRN2", target_bir_lowering=False, debug=False,
                   num_devices=NC, num_swdge_queues=4)

    ein = lambda n, s, d=dt.float32: nc.dram_tensor(n, s, d, kind="ExternalInput")
    xT = ein("xT", [17, TI * 128], dt.bfloat16)
    xnT = ein("xnT", [9, TN * 128], dt.bfloat16)
    disf = ein("disf", [128, NT]); disr = ein("disr", [128, NT])
    invf = ein("invf", [128, NT]); invr = ein("invr", [128, NT])
    gidx_d = [ein("gidx_f", [128, Ss[0] // 16], dt.int16),
              ein("gidx_r", [128, Ss[1] // 16], dt.int16)]
    ind_d = [ein("ind_f", [128, Ss[0]], dt.float8e4),
             ein("ind_r", [128, Ss[1]], dt.float8e4)]
    enc1_Wb = ein("enc1_Wb", [17, 128], dt.bfloat16)
    enc2_W = ein("enc2_W", [128, 64], dt.bfloat16)
    enc2_b = ein("enc2_b", [64, 1])
    net1_Wb = ein("net1_Wb", [9, 64], dt.bfloat16)
    net2_W = ein("net2_W", [64, 64], dt.bfloat16)
    net2_b = ein("net2_b", [64, 1])
    wcat_d = ein("wcat", [L, 65, 128], dt.bfloat16)
    wcatr_d = ein("wcat_root", [L, 65, 128], dt.bfloat16)
    ln_g_t = ein("ln_g", [L, 128, 64]); ln_b_t = ein("ln_b", [L, 128, 64])
    out = nc.dram_tensor("out", [DST, (L + 1) * D], dt.float32,
                         kind="ExternalOutput")

    xp_own = nc.dram_tensor("xp_own", [TAB, 128], dt.bfloat16)
    xp_full = nc.dram_tensor("xp_full", [NC * TAB, 128], dt.bfloat16,
                             addr_space="Shared")

    with tile.TileContext(nc) as tc:
        with (
            tc.tile_pool(name="const", bufs=1) as cpool,
            tc.tile_pool(name="enc", bufs=3) as epool,
            tc.tile_pool(name="idx", bufs=4) as ipool,
            tc.tile_pool(name="dlc", bufs=4) as dpool,
            tc.tile_pool(name="gat", bufs=6) as gpool,
            tc.tile_pool(name="ind", bufs=6) as npool,
            tc.tile_pool(name="cmb", bufs=4) as mpool,
            tc.tile_pool(name="hst", bufs=WT + 3) as hpool,
            tc.tile_pool(name="sml", bufs=3) as spool,
            tc.tile_pool(name="xps", bufs=3) as xpool,
            tc.tile_pool(name="agg", bufs=2, space="PSUM") as apool,
            tc.tile_pool(name="pmm", bufs=2, space="PSUM") as mmpool,
        ):
            # ---------- constants ----------
            h_res = cpool.tile([65, NT * 128], dt.bfloat16)
            for o in range(0, NT * 128, 4096):
                ww = min(4096, NT * 128 - o)
                nc.vector.memset(h_res[64:65, o:o + ww], 1.0)
            dfo = cpool.tile([128, NT], dt.float32)
            dro = cpool.tile([128, NT], dt.float32)
            ifo = cpool.tile([128, NT], dt.float32)
            iro = cpool.tile([128, NT], dt.float32)
            nc.sync.dma_start(out=dfo[:], in_=disf[:, :])
            nc.sync.dma_start(out=dro[:], in_=disr[:, :])
            nc.sync.dma_start(out=ifo[:], in_=invf[:, :])
            nc.sync.dma_start(out=iro[:], in_=invr[:, :])
            e1w = cpool.tile([17, 128], dt.bfloat16)
            e2w = cpool.tile([128, 64], dt.bfloat16)
            e2b = cpool.tile([64, 1], dt.float32)
            n1w = cpool.tile([9, 64], dt.bfloat16)
            n2w = cpool.tile([64, 64], dt.bfloat16)
            n2b = cpool.tile([64, 1], dt.float32)
            nc.sync.dma_start(out=e1w[:], in_=enc1_Wb[:, :])
            nc.sync.dma_start(out=e2w[:], in_=enc2_W[:, :])
            nc.sync.dma_start(out=e2b[:], in_=enc2_b[:, :])
            nc.sync.dma_start(out=n1w[:], in_=net1_Wb[:, :])
            nc.sync.dma_start(out=n2w[:], in_=net2_W[:, :])
            nc.sync.dma_start(out=n2b[:], in_=net2_b[:, :])
            wc_sb = cpool.tile([65, L * 128], dt.bfloat16)
            wcr_sb = cpool.tile([65, L * 128], dt.bfloat16)
            nc.sync.dma_start(
                out=wc_sb[:].rearrange("p (l d) -> p l d", l=L),
                in_=wcat_d.ap().rearrange("l p d -> p l d"))
            nc.sync.dma_start(
                out=wcr_sb[:].rearrange("p (l d) -> p l d", l=L),
                in_=wcatr_d.ap().rearrange("l p d -> p l d"))
            lng_sb = cpool.tile([128, L * 64], dt.float32)
            lnb_sb = cpool.tile([128, L * 64], dt.float32)
            nc.sync.dma_start(
                out=lng_sb[:].rearrange("p (l d) -> p l d", l=L),
                in_=ln_g_t.ap().rearrange("l p d -> p l d"))
            nc.sync.dma_start(
                out=lnb_sb[:].rearrange("p (l d) -> p l d", l=L),
                in_=ln_b_t.ap().rearrange("l p d -> p l d"))
            ident32 = cpool.tile([128, 128], dt.float32)
            _masks.make_identity(nc, ident32[:])
            ident16 = cpool.tile([64, 64], dt.bfloat16)
            _masks.make_identity(nc, ident16[:])
            # pre-zero the gather pool bufs: pad slots use negative (skipped)
            # gather indices, so whatever is in the buffer must be finite for
            # the 0-weighted matmul contribution to stay 0.
            for _z in range(6):
                gz = gpool.tile([128, CALL_MAX // 128, 128], dt.bfloat16,
                                tag="gat", name=f"gz{_z}")
                nc.vector.memset(gz[:], 0.0)

            def leaky(dst_ap, src_ap, tmp_tile):
                nc.vector.tensor_scalar(out=tmp_tile, in0=src_ap, scalar1=0.1,
                                        scalar2=None, op0=OP.mult)
                nc.vector.tensor_tensor(out=dst_ap, in0=src_ap, in1=tmp_tile,
                                        op=OP.max)

            # ---------- x' phase for one tile ----------
            def emit_xphase(l, t):
                px = mmpool.tile([128, 512], dt.float32, tag="mm")
                nc.tensor.matmul(px[:, :128],
                                 h_res[:, t * 128:(t + 1) * 128],
                                 wc_sb[:, l * 128:(l + 1) * 128],
                                 start=True, stop=True)
                xps = xpool.tile([128, 128], dt.bfloat16, tag="xps")
                nc.scalar.activation(out=xps[:, 0:64], in_=px[:, 0:64],
                                     func=AF.Relu, scale=dfo[:, t:t + 1])
                nc.scalar.activation(out=xps[:, 64:128], in_=px[:, 64:128],
                                     func=AF.Relu, scale=dro[:, t:t + 1])
                nc.sync.dma_start(out=xp_own[t * TP:(t + 1) * TP, :],
                                  in_=xps[0:TP, :])

            # ---------- encoder ----------
            def encode(inpT, w1, nf1, nmid, w2, b2, ncols, col_base):
                for t0 in range(0, ncols, 512):
                    w = min(512, ncols - t0)
                    rhs = epool.tile([nf1, 512], dt.bfloat16, tag="erhs")
                    nc.sync.dma_start(out=rhs[:, :w], in_=inpT[:, t0:t0 + w])
                    p1 = mmpool.tile([128, 512], dt.float32, tag="mm")
                    nc.tensor.matmul(p1[:nmid, :w], w1[:], rhs[:nf1, :w],
                                     start=True, stop=True)
                    s1 = epool.tile([128, 512], dt.bfloat16, tag="es1")
                    tmp1 = epool.tile([128, 512], dt.float32, tag="etmp1")
                    leaky(s1[:nmid, :w], p1[:nmid, :w], tmp1[:nmid, :w])
                    p2 = mmpool.tile([128, 512], dt.float32, tag="mm")
                    nc.tensor.matmul(p2[:64, :w], w2[:], s1[:nmid, :w],
                                     start=True, stop=True)
                    s2 = epool.tile([64, 512], dt.bfloat16, tag="es2")
                    badd = epool.tile([64, 512], dt.float32, tag="ebadd")
                    nc.vector.tensor_scalar(out=badd[:, :w], in0=p2[:64, :w],
                                            scalar1=b2[:, 0:1], scalar2=None,
                                            op0=OP.add)
                    tmp2 = epool.tile([64, 512], dt.float32, tag="etmp2")
                    leaky(s2[:, :w], badd[:, :w], tmp2[:, :w])
                    nc.vector.tensor_copy(
                        out=h_res[0:64, col_base + t0:col_base + t0 + w],
                        in_=s2[:, :w])
                    for m0 in range(0, w, 128):
                        mw = min(128, w - m0)
                        pt = mmpool.tile([128, 512], dt.bfloat16, tag="mm")
                        nc.tensor.matmul(pt[:mw, :64], s2[:, m0:m0 + mw],
                                         ident16[:, :], start=True, stop=True,
                                         is_transpose=True)
                        hc = epool.tile([128, 64], dt.float32, tag="ehc")
                        nc.scalar.activation(out=hc[:mw, :], in_=pt[:mw, :64],
                                             func=AF.Copy)
                        nc.sync.dma_start(
                            out=out[col_base + t0 + m0:col_base + t0 + m0 + mw,
                                    0:64],
                            in_=hc[:mw, :])

            encode(xT, e1w, 17, 128, e2w, e2b, TI * 128, 0)
            encode(xnT, n1w, 9, 64, n2w, n2b, TN * 128, TI * 128)

            for t in range(NT):
                emit_xphase(0, t)

            # ---------- layers ----------
            qn = 0
            for l in range(L):
                nc.gpsimd.collective_compute(
                    "AllGather", OP.bypass,
                    replica_groups=[list(range(NC))],
                    ins=[xp_own.ap().opt()], outs=[xp_full.ap().opt()])

                use_g = flags["ln_g"][l]
                use_b = flags["ln_b"][l]
                offs = [0, 0]
                for w in range(NW):
                    tiles = list(range(w * WT, min(NT, (w + 1) * WT)))
                    aggT = [apool.tile([128, 512], dt.float32, tag=f"agg{j}",
                                       name=f"agg{j}")
                            for j in range(3)]

                    def agg_slice(i, d):
                        slot = i * 2 + d
                        return aggT[slot // 8][:, (slot % 8) * 64:
                                               (slot % 8) * 64 + 64]

                    # one PSUM accumulation group per bank (zero regions are
                    # 2KB): first matmul into a bank starts the group (which
                    # marks the whole bank pending-zero), the last stops it.
                    seq = []
                    for dd in range(2):
                        gsz = gszs[dd]
                        for s in range(NC):
                            for i, t in enumerate(tiles):
                                for k2 in range(int(gsz[t, s]) // 128):
                                    seq.append(((i * 2 + dd) // 8,
                                                (dd, s, i, k2)))
                    first_mm = {}
                    last_mm = {}
                    for j, kk in seq:
                        if j not in first_mm:
                            first_mm[j] = kk
                        last_mm[j] = kk
                    first_mm = set(first_mm.values())
                    last_mm = set(last_mm.values())

                    for dd in range(2):
                        gsz = gszs[dd]
                        for s in range(NC):
                            u0 = offs[dd]
                            usz = int(gsz[tiles[0]:tiles[-1] + 1, s].sum())
                            it = dpool.tile([128, 8192], dt.float8e4,
                                            tag="indt")
                            nc.sync.dma_start(
                                out=it[:, :usz],
                                in_=ind_d[dd][:, u0:u0 + usz])
                            git = ipool.tile([128, 512], dt.int16, tag="git")
                            nc.sync.dma_start(
                                out=git[:, :usz // 16],
                                in_=gidx_d[dd][:, u0 // 16:(u0 + usz) // 16])
                            # gather calls
                            gts = []
                            k = 0
                            while k < usz:
                                csz = min(CALL_MAX, usz - k)
                                gt = gpool.tile([128, CALL_MAX // 128, 128],
                                                dt.bfloat16, tag="gat")
                                nc.gpsimd.dma_gather(
                                    out_ap=gt[:, :csz // 128, :],
                                    in_ap=xp_full[s * TAB:(s + 1) * TAB, :],
                                    idxs_ap=git[:, k // 16:(k + csz) // 16],
                                    num_idxs=csz, num_idxs_reg=csz,
                                    elem_size=128, elem_step=128,
                                    queue_num=qn % 4)
                                qn += 1
                                gts.append((k, csz, gt))
                                k += csz
                            # matmuls
                            rel = 0
                            for i, t in enumerate(tiles):
                                n128 = int(gsz[t, s]) // 128
                                for k2 in range(n128):
                                    r = rel + k2 * 128
                                    ci = r // CALL_MAX
                                    k0, csz0, gt0 = gts[ci]
                                    kk = (dd, s, i, k2)
                                    nc.tensor.matmul(
                                        agg_slice(i, dd),
                                        it[:, r:r + 128],
                                        gt0[:, (r - k0) // 128,
                                            dd * 64:dd * 64 + 64],
                                        start=(kk in first_mm),
                                        stop=(kk in last_mm))
                                rel += n128 * 128
                            offs[dd] += usz

                    # ----- combine wave -----
                    nwt = len(tiles)
                    sums = spool.tile([128, WT], dt.float32, tag="sums")
                    sqs = spool.tile([128, WT], dt.float32, tag="sqs")
                    hss = []
                    for i, t in enumerate(tiles):
                        p2 = mmpool.tile([128, 512], dt.float32, tag="mm")
                        nc.tensor.matmul(p2[:, :128],
                                         h_res[:, t * 128:(t + 1) * 128],
                                         wcr_sb[:, l * 128:(l + 1) * 128],
                                         start=True, stop=True)
                        stf = mpool.tile([128, 64], dt.float32, tag="stf")
                        stv = mpool.tile([128, 64], dt.float32, tag="stv")
                        nc.scalar.activation(out=stf[:], in_=p2[:, 0:64],
                                             func=AF.Relu,
                                             scale=ifo[:, t:t + 1])
                        nc.scalar.activation(out=stv[:], in_=p2[:, 64:128],
                                             func=AF.Relu,
                                             scale=iro[:, t:t + 1])
                        af = mpool.tile([128, 64], dt.float32, tag="af")
                        ar = mpool.tile([128, 64], dt.float32, tag="ar")
                        nc.scalar.activation(out=af[:], in_=agg_slice(i, 0),
                                             func=AF.Copy,
                                             scale=dfo[:, t:t + 1])
                        nc.scalar.activation(out=ar[:], in_=agg_slice(i, 1),
                                             func=AF.Copy,
                                             scale=dro[:, t:t + 1])
                        h1 = mpool.tile([128, 64], dt.float32, tag="h1")
                        h2 = mpool.tile([128, 64], dt.float32, tag="h2")
                        hs = hpool.tile([128, 64], dt.float32, tag="hs")
                        nc.vector.tensor_tensor(out=h1[:], in0=af[:],
                                                in1=stf[:], op=OP.add)
                        nc.vector.tensor_tensor(out=h2[:], in0=ar[:],
                                                in1=stv[:], op=OP.add)
                        nc.vector.tensor_tensor(out=hs[:], in0=h1[:],
                                                in1=h2[:], op=OP.add)
                        sc1 = mpool.tile([128, 64], dt.float32, tag="sc1")
                        sc2 = mpool.tile([128, 64], dt.float32, tag="sc2")
                        nc.scalar.activation(out=sc1[:], in_=hs[:],
                                             func=AF.Identity,
                                             accum_out=sums[:, i:i + 1])
                        nc.scalar.activation(out=sc2[:], in_=hs[:],
                                             func=AF.Square,
                                             accum_out=sqs[:, i:i + 1])
                        hss.append(hs)
                    m8 = spool.tile([128, WT], dt.float32, tag="m8")
                    ex2 = spool.tile([128, WT], dt.float32, tag="ex2")
                    nc.vector.tensor_scalar(out=m8[:, :nwt], in0=sums[:, :nwt],
                                            scalar1=1.0 / 64, scalar2=None,
                                            op0=OP.mult)
                    nc.vector.tensor_scalar(out=ex2[:, :nwt], in0=sqs[:, :nwt],
                                            scalar1=1.0 / 64, scalar2=None,
                                            op0=OP.mult)
                    msq = spool.tile([128, WT], dt.float32, tag="msq")
                    nc.vector.tensor_tensor(out=msq[:, :nwt], in0=m8[:, :nwt],
                                            in1=m8[:, :nwt], op=OP.mult)
                    var = spool.tile([128, WT], dt.float32, tag="var")
                    nc.vector.tensor_tensor(out=var[:, :nwt], in0=ex2[:, :nwt],
                                            in1=msq[:, :nwt], op=OP.subtract)
                    vpe = spool.tile([128, WT], dt.float32, tag="vpe")
                    nc.vector.tensor_scalar(out=vpe[:, :nwt], in0=var[:, :nwt],
                                            scalar1=EPS, scalar2=None,
                                            op0=OP.add)
                    sd = spool.tile([128, WT], dt.float32, tag="sd")
                    nc.scalar.activation(out=sd[:, :nwt], in_=vpe[:, :nwt],
                                         func=AF.Sqrt)
                    rstd = spool.tile([128, WT], dt.float32, tag="rstd")
                    nc.vector.reciprocal(out=rstd[:, :nwt], in_=sd[:, :nwt])
                    nmr = spool.tile([128, WT], dt.float32, tag="nmr")
                    nc.vector.tensor_scalar(out=nmr[:, :nwt], in0=m8[:, :nwt],
                                            scalar1=-1.0, scalar2=None,
                                            op0=OP.mult)
                    nc.vector.tensor_tensor(out=nmr[:, :nwt], in0=nmr[:, :nwt],
                                            in1=rstd[:, :nwt], op=OP.mult)
                    for i, t in enumerate(tiles):
                        hs = hss[i]
                        nm = mpool.tile([128, 64], dt.float32, tag="nm")
                        nc.scalar.activation(out=nm[:], in_=hs[:],
                                             func=AF.Identity,
                                             scale=rstd[:, i:i + 1],
                                             bias=nmr[:, i:i + 1])
                        cur = nm
                        if use_g:
                            gm = mpool.tile([128, 64], dt.float32, tag="gm")
                            nc.vector.tensor_tensor(
                                out=gm[:], in0=cur[:],
                                in1=lng_sb[:, l * 64:(l + 1) * 64], op=OP.mult)
                            cur = gm
                        if use_b:
                            bm = mpool.tile([128, 64], dt.float32, tag="bm")
                            nc.vector.tensor_tensor(
                                out=bm[:], in0=cur[:],
                                in1=lnb_sb[:, l * 64:(l + 1) * 64], op=OP.add)
                            cur = bm
                        hn = mpool.tile([128, 64], dt.float32, tag="hn")
                        ltmp = mpool.tile([128, 64], dt.float32, tag="ltmp")
                        leaky(hn[:], cur[:], ltmp[:])
                        nc.sync.dma_start(
                            out=out[t * 128:(t + 1) * 128,
                                    (l + 1) * 64:(l + 2) * 64],
                            in_=hn[:])
                        if l < L - 1:
                            pt = mmpool.tile([128, 512], dt.float32, tag="mm")
                            nc.tensor.matmul(pt[:64, :128], hn[:],
                                             ident32[:, :], start=True,
                                             stop=True, is_transpose=True)
                            nc.scalar.activation(
                                out=h_res[0:64, t * 128:(t + 1) * 128],
                                in_=pt[:64, :128], func=AF.Copy)
                            emit_xphase(l + 1, t)

    nc.compile()
    return nc


# ---------------------------------------------------------------------------
# entry point
# ---------------------------------------------------------------------------

def kernel(**inputs):
    from concourse.bass_utils import run_bass_kernel_spmd

    meta, per_core = _prep(inputs)
    key = (meta["n_inst"], meta["n_net"], meta["S_f"], meta["S_r"],
           meta["gsz_f"].tobytes(), meta["gsz_r"].tobytes(),
           tuple(meta["flags"]["ln_g"]), tuple(meta["flags"]["ln_b"]))
    if key not in _CACHE:
        _CACHE.clear()
        _CACHE[key] = _build(meta)
    nc = _CACHE[key]

    res = run_bass_kernel_spmd(nc, per_core, core_ids=list(range(NC)))

    n_inst, n_net = meta["n_inst"], meta["n_net"]
    si, sn = meta["si"], meta["sn"]
    outp = np.empty((n_inst + n_net, (L + 1) * D), np.float32)
    ji = np.arange(si)
    ri = (ji % TI) * 128 + ji // TI
    jn = np.arange(sn)
    rn = (TI + jn % TN) * 128 + jn // TN
    for c in range(NC):
        oc = res.results[c]["out"]
        outp[c * si:(c + 1) * si] = oc[ri]
        outp[n_inst + c * sn:n_inst + (c + 1) * sn] = oc[rn]
    return outp
